# revision 1
# baseline (speedup 1.0000x reference)
"""AttentionDecoder (topk_masking) Trainium2 kernel.

Algorithm (matches the jax reference):
  combined = cat([current, context])           # [1, 2E]
  query    = combined @ Wq.T                   # [1, H]
  scores   = query @ Wk @ cand.T               # [1, N]  (keys folded into w)
  masked softmax -> top-50 filter -> renormalize -> log probs + categorical
  sample (jax key 42).

Distribution: candidates are sharded row-wise over 8 NeuronCores (25000
rows/core, zero-padded to 25088 = 196*128).  Each core receives its shard
pre-transposed (candT: [128(E), 25088]) and computes, fully on-device:
  * scores = cand_shard @ w   (196 PE matmuls, candidate tile stationary)
  * masking (additive -1e9 bias)
  * per-partition max and masked sum(exp(s - max)) (partial softmax stats)
The host gathers per-core masked scores + softmax partials, merges the
softmax statistics, selects the global top-50 and assembles the outputs.

The heavy data movement (102.4 MB of candidate embeddings) happens on the
8 cores; the kernel is memory-bound on the HBM read of the shard.
"""

import numpy as np

E = 128
N_TOTAL = 200000
NCORES = 8
SHARD = N_TOTAL // NCORES       # 25000 rows per core
NSLICE = 196                    # 128-wide score slices per core
PAD = NSLICE * 128              # 25088 padded rows per core
CHUNK = 3584                    # candT DMA chunk width (1.75 MB per chunk)
NCHUNK = PAD // CHUNK           # 7
MASKVAL = np.float32(-1.0e9)    # additive mask bias (exp underflows to 0)
TOPK = 50

_NC_CACHE = {}
LAST_RESULTS = None  # BassKernelResults of the most recent run (for profiling)


def _build_nc():
    """Build the per-core Bass program (identical on all 8 cores)."""
    import concourse.bacc as bacc
    import concourse.tile as tile
    from concourse import mybir

    f32 = mybir.dt.float32
    nc = bacc.Bacc(
        "TRN2",
        target_bir_lowering=False,
        debug=False,
        enable_asserts=False,
        num_devices=NCORES,
    )

    candT = nc.dram_tensor("candT", [128, PAD], f32, kind="ExternalInput")
    wcol = nc.dram_tensor("wcol", [128, 1], f32, kind="ExternalInput")
    mask01 = nc.dram_tensor("mask01", [128, NSLICE], f32, kind="ExternalInput")
    maskbias = nc.dram_tensor("maskbias", [128, NSLICE], f32, kind="ExternalInput")
    # outbuf[:, :196] = masked scores, [:, 196] = per-partition max,
    # [:, 197] = per-partition sum(exp(s - max)) over unmasked entries.
    outb = nc.dram_tensor("outbuf", [128, NSLICE + 2], f32, kind="ExternalOutput")

    with tile.TileContext(nc) as tc:
        with (
            tc.tile_pool(name="wpool", bufs=1) as wpool,
            tc.tile_pool(name="data", bufs=NCHUNK) as dpool,
            tc.tile_pool(name="aux", bufs=1) as apool,
            tc.tile_pool(name="ps", bufs=1, space="PSUM") as ppool,
        ):
            w_sb = wpool.tile([128, 1], f32, tag="w")
            nc.sync.dma_start(w_sb[:], wcol[:])

            chunks = []
            for c in range(NCHUNK):
                t = dpool.tile([128, CHUNK], f32, tag="chunk", name=f"chunk{c}")
                nc.sync.dma_start(t[:], candT[:, c * CHUNK : (c + 1) * CHUNK])
                chunks.append(t)

            m01_sb = apool.tile([128, NSLICE], f32, tag="m01")
            nc.sync.dma_start(m01_sb[:], mask01[:])
            mb_sb = apool.tile([128, NSLICE], f32, tag="mb")
            nc.sync.dma_start(mb_sb[:], maskbias[:])

            # scores: for slice s, out[p, s] = sum_e cand[s*128+p, e] * w[e]
            ps = ppool.tile([128, NSLICE], f32, tag="ps")
            spc = CHUNK // 128
            for c in range(NCHUNK):
                for si in range(spc):
                    s = c * spc + si
                    nc.tensor.matmul(
                        ps[:, s : s + 1],
                        chunks[c][:, si * 128 : (si + 1) * 128],
                        w_sb[:],
                        start=True,
                        stop=True,
                    )

            outsb = apool.tile([128, NSLICE + 2], f32, tag="outsb")
            scores = outsb[:, 0:NSLICE]
            nc.vector.tensor_add(scores, ps[:], mb_sb[:])

            colmax = apool.tile([128, 1], f32, tag="colmax")
            nc.vector.reduce_max(colmax[:], scores, axis=mybir.AxisListType.X)
            negmax = apool.tile([128, 1], f32, tag="negmax")
            nc.vector.tensor_scalar_mul(negmax[:], colmax[:], -1.0)

            e_sb = apool.tile([128, NSLICE], f32, tag="e")
            nc.scalar.activation(
                e_sb[:],
                scores,
                mybir.ActivationFunctionType.Exp,
                bias=negmax[:],
                scale=1.0,
            )
            em_sb = apool.tile([128, NSLICE], f32, tag="em")
            nc.vector.tensor_mul(em_sb[:], e_sb[:], m01_sb[:])
            nc.vector.reduce_sum(
                outsb[:, NSLICE + 1 : NSLICE + 2], em_sb[:], axis=mybir.AxisListType.X
            )
            nc.vector.tensor_copy(outsb[:, NSLICE : NSLICE + 1], colmax[:])

            nc.sync.dma_start(outb[:], outsb[:])

    nc.finalize()
    return nc


def get_nc():
    if "nc" not in _NC_CACHE:
        _NC_CACHE["nc"] = _build_nc()
    return _NC_CACHE["nc"]


def make_in_maps(cand, w, mask_np):
    """Shard + lay out host inputs for the 8 cores."""
    wcol = np.ascontiguousarray(w.reshape(1, E).T)  # [128, 1]
    in_maps = []
    for c in range(NCORES):
        shard = cand[c * SHARD : (c + 1) * SHARD]  # [25000, 128]
        candT = np.zeros((128, PAD), np.float32)
        candT[:, :SHARD] = shard.T

        mflat = np.zeros(PAD, np.float32)
        mflat[:SHARD] = mask_np[0, c * SHARD : (c + 1) * SHARD]
        # layout [p, s] corresponds to local row s*128 + p
        m01 = np.ascontiguousarray(mflat.reshape(NSLICE, 128).T)
        mbias = np.ascontiguousarray(
            ((mflat - 1.0) * (-MASKVAL)).astype(np.float32).reshape(NSLICE, 128).T
        )
        in_maps.append(
            {"candT": candT, "wcol": wcol, "mask01": m01, "maskbias": mbias}
        )
    return in_maps


def kernel(current_node_emb, context_emb, candidate_node_embs, Wq, Wk, mask):
    global LAST_RESULTS
    from concourse.bass_utils import run_bass_kernel_spmd

    cur = np.asarray(current_node_emb, np.float32)
    ctxe = np.asarray(context_emb, np.float32)
    cand = np.ascontiguousarray(np.asarray(candidate_node_embs, np.float32))
    Wq_np = np.asarray(Wq, np.float32)
    Wk_np = np.asarray(Wk, np.float32)
    mask_np = np.asarray(mask)

    # tiny query projection; scores = w @ cand.T with w = (combined @ Wq.T) @ Wk
    combined = np.concatenate([cur, ctxe], axis=1)  # [1, 2E]
    query = (combined @ Wq_np.T).astype(np.float32)  # [1, H]
    w = (query @ Wk_np).astype(np.float32)  # [1, E]

    in_maps = make_in_maps(cand, w, mask_np)
    nc = get_nc()
    res = run_bass_kernel_spmd(nc, in_maps, list(range(NCORES)))
    LAST_RESULTS = res

    # ---- gather / merge ----
    all_scores = np.empty(N_TOTAL, np.float32)
    colmaxes = np.empty((NCORES, 128), np.float32)
    rowsums = np.empty((NCORES, 128), np.float32)
    for c in range(NCORES):
        ob = np.asarray(res.results[c]["outbuf"])  # [128, 198]
        all_scores[c * SHARD : (c + 1) * SHARD] = ob[:, :NSLICE].T.reshape(-1)[:SHARD]
        colmaxes[c] = ob[:, NSLICE]
        rowsums[c] = ob[:, NSLICE + 1]

    # merge softmax statistics (the "all-reduce" step, done at gather time)
    m_glob = np.float32(colmaxes.max())
    Z = np.float32(
        np.sum(rowsums.astype(np.float64) * np.exp(colmaxes.astype(np.float64) - float(m_glob)))
    )

    probs = (np.exp(all_scores - m_glob) / Z).astype(np.float32)  # [N]

    # top-50 threshold on probabilities, exactly like the reference
    th = np.partition(probs, N_TOTAL - TOPK)[N_TOTAL - TOPK]
    top_mask = probs >= th
    filtered = (probs * top_mask).astype(np.float32)
    S = filtered.sum(dtype=np.float32)
    filtered = (filtered / (S + np.float32(1e-10))).astype(np.float32)

    log_probs_all = np.log(filtered + np.float32(1e-10)).astype(np.float32)
    with np.errstate(divide="ignore"):
        logits = np.where(
            filtered > 0, np.log(filtered), np.float32(-np.inf)
        ).astype(np.float32)

    log_probs_all = log_probs_all.reshape(1, N_TOTAL)
    logits = logits.reshape(1, N_TOTAL)

    # categorical sample with jax key 42 (on host CPU, exact reference RNG)
    import jax

    cpu = jax.devices("cpu")[0]
    with jax.default_device(cpu):
        action_idx = np.asarray(
            jax.random.categorical(
                jax.random.key(42), jax.numpy.asarray(logits), axis=1
            )
        )
    log_prob_action = np.take_along_axis(logits, action_idx[:, None], axis=1)[:, 0]

    return log_probs_all, log_prob_action, action_idx


# revision 9
# speedup vs baseline: 1.6517x; 1.6517x over previous
"""AttentionDecoder (topk_masking) Trainium2 kernel.

Algorithm (matches the jax reference):
  combined = cat([current, context])           # [1, 2E]
  query    = combined @ Wq.T                   # [1, H]
  scores   = query @ Wk @ cand.T               # [1, N]  (keys folded into w)
  masked softmax -> top-50 filter -> renormalize -> log probs + categorical
  sample (jax key 42).

Distribution: candidates are sharded row-wise over 8 NeuronCores (25000
rows/core, zero-padded to 25088 = 196*128).  Each core receives its shard
pre-transposed (candT: [128(E), 25088]) and computes, fully on-device:
  * scores = cand_shard @ w   (196 PE matmuls, candidate tile stationary)
  * masking (additive -1e9 bias)
  * per-partition max and masked sum(exp(s - max)) (partial softmax stats)
The host gathers per-core masked scores + softmax partials, merges the
softmax statistics, selects the global top-50 and assembles the outputs.

The heavy data movement (102.4 MB of candidate embeddings) happens on the
8 cores; the kernel is memory-bound on the HBM read of the shard.

Matmul dtype: float32r (the PE's single-pass fp32 mode).  Plain float32
lowers to 2 LDWEIGHTS+MATMUL pairs per matmul and doubles PE time; f32r
requires rhs free size >= 2, so the w vector is shipped duplicated
([w, w]) and the score columns are read back with stride 2.
"""

import os

import numpy as np

E = 128
N_TOTAL = 200000
NCORES = 8
SHARD = N_TOTAL // NCORES       # 25000 rows per core
NSLICE = 196                    # 128-wide score slices per core
PAD = NSLICE * 128              # 25088 padded rows per core
CHUNK = 3584                    # candT DMA chunk width (1.75 MB per chunk)
NCHUNK = PAD // CHUNK           # 7
MASKVAL = np.float32(-1.0e9)    # additive mask bias (exp underflows to 0)
TOPK = 50

_NC_CACHE = {}
LAST_RESULTS = None  # BassKernelResults of the most recent run (for profiling)
LAST_SCORES = None  # gathered masked scores of the most recent run (diagnostics)
VARIANT = os.environ.get("KERNEL_VARIANT", "f32r")


def _build_nc():
    """Build the per-core Bass program (identical on all 8 cores)."""
    import concourse.bacc as bacc
    import concourse.tile as tile
    from concourse import mybir

    f32 = mybir.dt.float32
    use_r = VARIANT == "f32r"
    fmm = mybir.dt.float32r if use_r else f32
    nw = 2 if use_r else 1  # f32r matmul needs rhs free size >= 2

    nc = bacc.Bacc(
        "TRN2",
        target_bir_lowering=False,
        debug=False,
        enable_asserts=False,
        num_devices=NCORES,
    )

    candT = nc.dram_tensor("candT", [128, PAD], fmm, kind="ExternalInput")
    wcol = nc.dram_tensor("wcol", [128, nw], fmm, kind="ExternalInput")
    mask01 = nc.dram_tensor("mask01", [128, NSLICE], f32, kind="ExternalInput")
    maskbias = nc.dram_tensor("maskbias", [128, NSLICE], f32, kind="ExternalInput")
    # outbuf[:, :196] = masked scores, [:, 196] = per-partition max,
    # [:, 197] = per-partition sum(exp(s - max)) over unmasked entries.
    outb = nc.dram_tensor("outbuf", [128, NSLICE + 2], f32, kind="ExternalOutput")

    with tile.TileContext(nc) as tc:
        with (
            tc.tile_pool(name="wpool", bufs=1) as wpool,
            tc.tile_pool(name="data", bufs=NCHUNK) as dpool,
            tc.tile_pool(name="aux", bufs=1) as apool,
            tc.tile_pool(name="ps", bufs=1, space="PSUM") as ppool,
        ):
            w_sb = wpool.tile([128, nw], fmm, tag="w")
            nc.sync.dma_start(w_sb[:], wcol[:])

            chunks = []
            for c in range(NCHUNK):
                t = dpool.tile([128, CHUNK], fmm, tag="chunk", name=f"chunk{c}")
                nc.sync.dma_start(t[:], candT[:, c * CHUNK : (c + 1) * CHUNK])
                chunks.append(t)

            m01_sb = apool.tile([128, NSLICE], f32, tag="m01")
            nc.sync.dma_start(m01_sb[:], mask01[:])
            mb_sb = apool.tile([128, NSLICE], f32, tag="mb")
            nc.sync.dma_start(mb_sb[:], maskbias[:])

            # scores: for slice s, out[p, s] = sum_e cand[s*128+p, e] * w[e]
            ps = ppool.tile([128, NSLICE * nw], f32, tag="ps")
            spc = CHUNK // 128
            for c in range(NCHUNK):
                for si in range(spc):
                    s = c * spc + si
                    nc.tensor.matmul(
                        ps[:, s * nw : (s + 1) * nw],
                        chunks[c][:, si * 128 : (si + 1) * 128],
                        w_sb[:],
                        start=True,
                        stop=True,
                    )
            ps_scores = ps[:, 0 : NSLICE * nw : nw]  # [128, 196]

            outsb = apool.tile([128, NSLICE + 2], f32, tag="outsb")
            scores = outsb[:, 0:NSLICE]
            nc.vector.tensor_add(scores, ps_scores, mb_sb[:])

            colmax = apool.tile([128, 1], f32, tag="colmax")
            nc.vector.reduce_max(colmax[:], scores, axis=mybir.AxisListType.X)
            negmax = apool.tile([128, 1], f32, tag="negmax")
            nc.vector.tensor_scalar_mul(negmax[:], colmax[:], -1.0)

            e_sb = apool.tile([128, NSLICE], f32, tag="e")
            nc.scalar.activation(
                e_sb[:],
                scores,
                mybir.ActivationFunctionType.Exp,
                bias=negmax[:],
                scale=1.0,
            )
            em_sb = apool.tile([128, NSLICE], f32, tag="em")
            nc.vector.tensor_mul(em_sb[:], e_sb[:], m01_sb[:])
            nc.vector.reduce_sum(
                outsb[:, NSLICE + 1 : NSLICE + 2], em_sb[:], axis=mybir.AxisListType.X
            )
            nc.vector.tensor_copy(outsb[:, NSLICE : NSLICE + 1], colmax[:])

            nc.sync.dma_start(outb[:], outsb[:])

    nc.finalize()
    return nc


def get_nc():
    if "nc" not in _NC_CACHE:
        _NC_CACHE["nc"] = _build_nc()
    return _NC_CACHE["nc"]


def make_in_maps(cand, w, mask_np):
    """Shard + lay out host inputs for the 8 cores."""
    nw = 2 if VARIANT == "f32r" else 1
    wcol = np.ascontiguousarray(
        np.repeat(w.reshape(1, E).T, nw, axis=1)
    )  # [128, nw]
    in_maps = []
    for c in range(NCORES):
        shard = cand[c * SHARD : (c + 1) * SHARD]  # [25000, 128]
        candT = np.zeros((128, PAD), np.float32)
        candT[:, :SHARD] = shard.T

        mflat = np.zeros(PAD, np.float32)
        mflat[:SHARD] = mask_np[0, c * SHARD : (c + 1) * SHARD]
        # layout [p, s] corresponds to local row s*128 + p
        m01 = np.ascontiguousarray(mflat.reshape(NSLICE, 128).T)
        mbias = np.ascontiguousarray(
            ((mflat - 1.0) * (-MASKVAL)).astype(np.float32).reshape(NSLICE, 128).T
        )
        in_maps.append(
            {"candT": candT, "wcol": wcol, "mask01": m01, "maskbias": mbias}
        )
    return in_maps


def kernel(current_node_emb, context_emb, candidate_node_embs, Wq, Wk, mask):
    global LAST_RESULTS, LAST_SCORES
    from concourse.bass_utils import run_bass_kernel_spmd

    cur = np.asarray(current_node_emb, np.float32)
    ctxe = np.asarray(context_emb, np.float32)
    cand = np.ascontiguousarray(np.asarray(candidate_node_embs, np.float32))
    Wq_np = np.asarray(Wq, np.float32)
    Wk_np = np.asarray(Wk, np.float32)
    mask_np = np.asarray(mask)

    # tiny query projection; scores = w @ cand.T with w = (combined @ Wq.T) @ Wk
    combined = np.concatenate([cur, ctxe], axis=1)  # [1, 2E]
    query = (combined @ Wq_np.T).astype(np.float32)  # [1, H]
    w = (query @ Wk_np).astype(np.float32)  # [1, E]

    in_maps = make_in_maps(cand, w, mask_np)
    nc = get_nc()
    res = run_bass_kernel_spmd(nc, in_maps, list(range(NCORES)))
    LAST_RESULTS = res

    # ---- gather / merge ----
    all_scores = np.empty(N_TOTAL, np.float32)
    colmaxes = np.empty((NCORES, 128), np.float32)
    rowsums = np.empty((NCORES, 128), np.float32)
    for c in range(NCORES):
        ob = np.asarray(res.results[c]["outbuf"])  # [128, 198]
        all_scores[c * SHARD : (c + 1) * SHARD] = ob[:, :NSLICE].T.reshape(-1)[:SHARD]
        colmaxes[c] = ob[:, NSLICE]
        rowsums[c] = ob[:, NSLICE + 1]
    LAST_SCORES = all_scores

    # merge softmax statistics (the "all-reduce" step, done at gather time)
    m_glob = np.float32(colmaxes.max())
    Z = np.float32(
        np.sum(
            rowsums.astype(np.float64)
            * np.exp(colmaxes.astype(np.float64) - float(m_glob))
        )
    )

    probs = (np.exp(all_scores - m_glob) / Z).astype(np.float32)  # [N]

    # top-50 threshold on probabilities, exactly like the reference
    th = np.partition(probs, N_TOTAL - TOPK)[N_TOTAL - TOPK]
    top_mask = probs >= th
    filtered = (probs * top_mask).astype(np.float32)
    S = filtered.sum(dtype=np.float32)
    filtered = (filtered / (S + np.float32(1e-10))).astype(np.float32)

    log_probs_all = np.log(filtered + np.float32(1e-10)).astype(np.float32)
    with np.errstate(divide="ignore"):
        logits = np.where(filtered > 0, np.log(filtered), np.float32(-np.inf)).astype(
            np.float32
        )

    log_probs_all = log_probs_all.reshape(1, N_TOTAL)
    logits = logits.reshape(1, N_TOTAL)

    # categorical sample with jax key 42 (on host CPU, exact reference RNG)
    import jax

    cpu = jax.devices("cpu")[0]
    with jax.default_device(cpu):
        action_idx = np.asarray(
            jax.random.categorical(
                jax.random.key(42), jax.numpy.asarray(logits), axis=1
            )
        )
    log_prob_action = np.take_along_axis(logits, action_idx[:, None], axis=1)[:, 0]

    return log_probs_all, log_prob_action, action_idx


# revision 10
# speedup vs baseline: 1.9971x; 1.2091x over previous
"""AttentionDecoder (topk_masking) Trainium2 kernel.

Algorithm (matches the jax reference):
  combined = cat([current, context])           # [1, 2E]
  query    = combined @ Wq.T                   # [1, H]
  scores   = query @ Wk @ cand.T               # [1, N]  (keys folded into w)
  masked softmax -> top-50 filter -> renormalize -> log probs + categorical
  sample (jax key 42).

Distribution: candidates are sharded row-wise over 8 NeuronCores (25000
rows/core, zero-padded to 25088 = 196*128).  Each core receives its shard
pre-transposed and split into bf16 hi/lo halves (c = hi + lo, exact to
~2^-17 relative).  On device, per 128-candidate slice, two accumulating
PE matmuls (hi and lo candidate tiles stationary, rhs = [w_hi, w_lo])
produce all four cross terms in fp32 PSUM — a near-fp32-exact dot
product at bf16 matmul speed.  Masking (additive -1e9), and the partial
softmax statistics sum(exp(s - shift)) with a fixed shift, are computed
per chunk, overlapped with the next chunk's matmuls.

The host gathers the per-core masked scores + softmax partials, merges
the statistics (the "all-reduce" step), selects the top-60 by device
score, re-scores exactly those candidates on the reference fp32 path
(60 rows — negligible), and assembles the outputs; the softmax
normalizer cancels in the renormalized top-50 distribution, so the
outputs are fp32-exact.

The kernel is memory-bound on the HBM read of the 102.4 MB of candidate
embeddings (12.8 MB/core).
"""

import os

import numpy as np

E = 128
N_TOTAL = 200000
NCORES = 8
SHARD = N_TOTAL // NCORES       # 25000 rows per core
NSLICE = 196                    # 128-wide score slices per core
PAD = NSLICE * 128              # 25088 padded rows per core
NCHUNK = 7
SPC = NSLICE // NCHUNK          # 28 slices per chunk
CCOLS = SPC * 128               # 3584 candidate columns per chunk (per half)
MASKVAL = np.float32(-1.0e9)    # additive mask bias (exp underflows to 0)
TOPK = 50
RESCORE = 60                    # candidates re-scored exactly on host

_NC_CACHE = {}
LAST_RESULTS = None  # BassKernelResults of the most recent run (for profiling)
LAST_SCORES = None  # gathered masked scores of the most recent run (diagnostics)


def _build_nc():
    """Build the per-core Bass program (identical on all 8 cores)."""
    import concourse.bacc as bacc
    import concourse.tile as tile
    from concourse import mybir

    f32 = mybir.dt.float32
    bf16 = mybir.dt.bfloat16

    nc = bacc.Bacc(
        "TRN2",
        target_bir_lowering=False,
        debug=False,
        enable_asserts=False,
        num_devices=NCORES,
    )

    # per chunk c: cols [c*2*CCOLS, c*2*CCOLS+CCOLS) = hi, next CCOLS = lo
    candHL = nc.dram_tensor("candHL", [128, 2 * PAD], bf16, kind="ExternalInput")
    wcol = nc.dram_tensor("wcol", [128, 2], bf16, kind="ExternalInput")
    maskbias = nc.dram_tensor("maskbias", [128, NSLICE], f32, kind="ExternalInput")
    negshift = nc.dram_tensor("negshift", [128, 1], f32, kind="ExternalInput")
    # outbuf[:, :196] = masked scores, [:, 196] = per-partition
    # sum(exp(s - shift)) over unmasked entries
    outb = nc.dram_tensor("outbuf", [128, NSLICE + 1], f32, kind="ExternalOutput")

    with tile.TileContext(nc) as tc:
        with (
            tc.tile_pool(name="wpool", bufs=1) as wpool,
            tc.tile_pool(name="data", bufs=NCHUNK) as dpool,
            tc.tile_pool(name="aux", bufs=1) as apool,
            tc.tile_pool(name="tmp", bufs=2) as tpool,
            tc.tile_pool(name="ps", bufs=3, space="PSUM") as ppool,
        ):
            w_sb = wpool.tile([128, 2], bf16, tag="w")
            nc.gpsimd.dma_start(w_sb[:], wcol[:])
            mb_sb = apool.tile([128, NSLICE], f32, tag="mb")
            nc.gpsimd.dma_start(mb_sb[:], maskbias[:])
            ns_sb = apool.tile([128, 1], f32, tag="ns")
            nc.gpsimd.dma_start(ns_sb[:], negshift[:])

            chunks = []
            for c in range(NCHUNK):
                t = dpool.tile([128, 2 * CCOLS], bf16, tag="chunk", name=f"chunk{c}")
                nc.sync.dma_start(
                    t[:], candHL[:, c * 2 * CCOLS : (c + 1) * 2 * CCOLS]
                )
                chunks.append(t)

            esums = apool.tile([128, NCHUNK], f32, tag="esums")
            outsb = apool.tile([128, NSLICE + 1], f32, tag="outsb")

            for c in range(NCHUNK):
                hi = chunks[c][:, 0:CCOLS]
                lo = chunks[c][:, CCOLS : 2 * CCOLS]
                pst = ppool.tile([128, 2 * SPC], f32, tag="psc", name=f"psc{c}")
                # per slice: psum[:, 2si:2si+2] = [c@wh + ..., c@wl + ...]
                # (hi and lo accumulate; score = col0 + col1, all 4 cross
                # terms of (c_hi + c_lo) @ (w_hi + w_lo))
                for si in range(SPC):
                    nc.tensor.matmul(
                        pst[:, 2 * si : 2 * si + 2],
                        hi[:, si * 128 : (si + 1) * 128],
                        w_sb[:],
                        start=True,
                        stop=False,
                    )
                    nc.tensor.matmul(
                        pst[:, 2 * si : 2 * si + 2],
                        lo[:, si * 128 : (si + 1) * 128],
                        w_sb[:],
                        start=False,
                        stop=True,
                    )
                # masked scores for this chunk, then exp partials
                sc = outsb[:, c * SPC : (c + 1) * SPC]
                tmp = tpool.tile([128, SPC], f32, tag="tmp")
                nc.vector.tensor_add(
                    tmp[:], pst[:, 0 : 2 * SPC : 2], mb_sb[:, c * SPC : (c + 1) * SPC]
                )
                nc.vector.tensor_add(sc, pst[:, 1 : 2 * SPC : 2], tmp[:])
                et = tpool.tile([128, SPC], f32, tag="et")
                nc.scalar.activation(
                    et[:],
                    sc,
                    mybir.ActivationFunctionType.Exp,
                    bias=ns_sb[:],
                    scale=1.0,
                    accum_out=esums[:, c : c + 1],
                )

            nc.vector.reduce_sum(
                outsb[:, NSLICE : NSLICE + 1], esums[:], axis=mybir.AxisListType.X
            )
            nc.sync.dma_start(outb[:], outsb[:])

    nc.finalize()
    return nc


def get_nc():
    if "nc" not in _NC_CACHE:
        _NC_CACHE["nc"] = _build_nc()
    return _NC_CACHE["nc"]


def make_in_maps(cand, w, mask_np, shift):
    """Shard + lay out host inputs for the 8 cores."""
    import ml_dtypes

    bf16 = ml_dtypes.bfloat16

    wh = w.reshape(E).astype(bf16)
    wl = (w.reshape(E) - wh.astype(np.float32)).astype(bf16)
    wcol = np.ascontiguousarray(np.stack([wh, wl], axis=1))  # [128, 2] bf16

    negshift = np.full((128, 1), -shift, np.float32)

    candT = cand.T  # [128, N] view
    in_maps = []
    for c in range(NCORES):
        xc = np.zeros((128, PAD), np.float32)
        xc[:, :SHARD] = candT[:, c * SHARD : (c + 1) * SHARD]
        hi = xc.astype(bf16)
        lo = (xc - hi.astype(np.float32)).astype(bf16)
        candHL = np.empty((128, 2 * PAD), bf16)
        for k in range(NCHUNK):
            candHL[:, k * 2 * CCOLS : k * 2 * CCOLS + CCOLS] = hi[
                :, k * CCOLS : (k + 1) * CCOLS
            ]
            candHL[:, k * 2 * CCOLS + CCOLS : (k + 1) * 2 * CCOLS] = lo[
                :, k * CCOLS : (k + 1) * CCOLS
            ]

        mflat = np.zeros(PAD, np.float32)
        mflat[:SHARD] = mask_np[0, c * SHARD : (c + 1) * SHARD]
        # layout [p, s] corresponds to local row s*128 + p
        mbias = np.ascontiguousarray(
            ((mflat - 1.0) * (-MASKVAL)).astype(np.float32).reshape(NSLICE, 128).T
        )
        in_maps.append(
            {
                "candHL": candHL,
                "wcol": wcol,
                "maskbias": mbias,
                "negshift": negshift,
            }
        )
    return in_maps


def kernel(current_node_emb, context_emb, candidate_node_embs, Wq, Wk, mask):
    global LAST_RESULTS, LAST_SCORES
    from concourse.bass_utils import run_bass_kernel_spmd

    cur = np.asarray(current_node_emb, np.float32)
    ctxe = np.asarray(context_emb, np.float32)
    cand = np.ascontiguousarray(np.asarray(candidate_node_embs, np.float32))
    Wq_np = np.asarray(Wq, np.float32)
    Wk_np = np.asarray(Wk, np.float32)
    mask_np = np.asarray(mask)

    # tiny query projection; scores = w @ cand.T with w = (combined @ Wq.T) @ Wk
    combined = np.concatenate([cur, ctxe], axis=1)  # [1, 2E]
    query = (combined @ Wq_np.T).astype(np.float32)  # [1, H]
    w = (query @ Wk_np).astype(np.float32)  # [1, E]

    # fixed exp shift: safe upper bound on any score
    shift = float(max(40.0, 16.0 * np.linalg.norm(w)))

    in_maps = make_in_maps(cand, w, mask_np, shift)
    nc = get_nc()
    res = run_bass_kernel_spmd(nc, in_maps, list(range(NCORES)))
    LAST_RESULTS = res

    # ---- gather / merge ----
    all_scores = np.empty(N_TOTAL, np.float32)
    rowsums = np.empty((NCORES, 128), np.float64)
    for c in range(NCORES):
        ob = np.asarray(res.results[c]["outbuf"])  # [128, 197]
        all_scores[c * SHARD : (c + 1) * SHARD] = ob[:, :NSLICE].T.reshape(-1)[:SHARD]
        rowsums[c] = ob[:, NSLICE]
    LAST_SCORES = all_scores

    # top-RESCORE candidates by device score; re-score them exactly on the
    # reference fp32 path (keys = cand @ Wk.T, s = query @ keys.T)
    sel = np.argpartition(all_scores, N_TOTAL - RESCORE)[N_TOTAL - RESCORE :]
    keys_sel = (cand[sel] @ Wk_np.T).astype(np.float32)  # [R, H]
    s_sel = (query @ keys_sel.T).astype(np.float32)[0]  # [R]

    # merge softmax statistics (the "all-reduce" step, done at gather time)
    m = np.float32(s_sel.max())
    Z = np.float32(np.exp(np.float64(shift) - np.float64(m)) * rowsums.sum())

    # exact probabilities of the re-scored candidates; top-50 threshold in
    # probability space, exactly like the reference
    p_sel = (np.exp(s_sel - m) / Z).astype(np.float32)
    th = np.sort(p_sel)[-TOPK]
    keep = p_sel >= th
    p_top = p_sel * keep
    S = p_top.sum(dtype=np.float32)
    fil_top = (p_top / (S + np.float32(1e-10))).astype(np.float32)

    log_probs_all = np.full(
        (1, N_TOTAL), np.log(np.float32(1e-10)), np.float32
    )
    logits = np.full((1, N_TOTAL), -np.inf, np.float32)
    sel_keep = sel[keep]
    fil_keep = fil_top[keep]
    log_probs_all[0, sel_keep] = np.log(fil_keep + np.float32(1e-10))
    logits[0, sel_keep] = np.log(fil_keep)

    # categorical sample with jax key 42 (on host CPU, exact reference RNG)
    import jax

    cpu = jax.devices("cpu")[0]
    with jax.default_device(cpu):
        action_idx = np.asarray(
            jax.random.categorical(
                jax.random.key(42), jax.numpy.asarray(logits), axis=1
            )
        )
    log_prob_action = np.take_along_axis(logits, action_idx[:, None], axis=1)[:, 0]

    return log_probs_all, log_prob_action, action_idx


# revision 24
# speedup vs baseline: 2.0786x; 1.0408x over previous
"""AttentionDecoder (topk_masking) Trainium2 kernel.

Algorithm (matches the jax reference):
  combined = cat([current, context])           # [1, 2E]
  query    = combined @ Wq.T                   # [1, H]
  scores   = query @ Wk @ cand.T               # [1, N]  (keys folded into w)
  masked softmax -> top-50 filter -> renormalize -> log probs + categorical
  sample (jax key 42).

Distribution: candidates are sharded row-wise over 8 NeuronCores (25000
rows/core, zero-padded to 25088 = 196*128).  Each core receives its shard
pre-transposed and split into bf16 hi/lo halves (c = hi + lo, exact to
~2^-17 relative).  On device, per 128-candidate slice, two accumulating
PE matmuls (hi and lo candidate tiles stationary, rhs = [w_hi, w_lo])
produce all four cross terms in fp32 PSUM — a near-fp32-exact dot
product at bf16 matmul speed.  Masking (additive -1e9) and the partial
softmax statistics sum(exp(s - shift)) with a fixed shift are computed
per chunk on DVE/ACT, overlapped with the next chunk's matmuls.

The kernel is written in raw Bass (hand-placed semaphores, no Tile
scheduler) so the only fixed overhead is the NRT preamble — the Tile
exit-barrier butterfly (~10 us) is avoided.  The w vector rides in the
first columns of the big candidate tensor so the PE can start as soon
as chunk 0 lands.

The host gathers the per-core masked scores + softmax partials, merges
the statistics (the "all-reduce" step), selects the top-60 by device
score, re-scores exactly those candidates on the reference fp32 path
(60 rows — negligible), and assembles the outputs; the softmax
normalizer cancels in the renormalized top-50 distribution, so the
outputs are fp32-exact.

The kernel is memory-bound on the HBM read of the 102.4 MB of candidate
embeddings (12.8 MB/core).
"""

import os

import numpy as np

E = 128
N_TOTAL = 200000
NCORES = 8
SHARD = N_TOTAL // NCORES       # 25000 rows per core
NSLICE = 196                    # 128-wide score slices per core
PAD = NSLICE * 128              # 25088 padded rows per core
NCHUNK = 7
SPC = NSLICE // NCHUNK          # 28 slices per chunk
CCOLS = SPC * 128               # 3584 candidate columns per chunk (per half)
CH2 = 2 * CCOLS                 # 7168 bf16 columns per chunk (hi + lo)
MASKVAL = np.float32(-1.0e9)    # additive mask bias (exp underflows to 0)
TOPK = 50
RESCORE = 60                    # candidates re-scored exactly on host

_NC_CACHE = {}
LAST_RESULTS = None  # BassKernelResults of the most recent run (for profiling)
LAST_SCORES = None  # gathered masked scores of the most recent run (diagnostics)


def _build_nc():
    """Raw-Bass per-core program (identical on all 8 cores)."""
    import concourse.bacc as bacc
    from concourse import mybir

    f32 = mybir.dt.float32
    bf16 = mybir.dt.bfloat16
    X = mybir.AxisListType.X
    Exp = mybir.ActivationFunctionType.Exp

    nc = bacc.Bacc(
        "TRN2",
        target_bir_lowering=False,
        debug=False,
        enable_asserts=False,
        num_devices=NCORES,
    )

    # layout: [w(2) | chunk0(hi 3584 | lo 3584) | chunk1 | ... | chunk6]
    candWHL = nc.dram_tensor(
        "candWHL", [128, 2 + 2 * PAD], bf16, kind="ExternalInput"
    )
    maskbias = nc.dram_tensor("maskbias", [128, NSLICE], f32, kind="ExternalInput")
    negshift = nc.dram_tensor("negshift", [128, 1], f32, kind="ExternalInput")
    # outbuf[:, :196] = masked scores, [:, 196] = per-partition
    # sum(exp(s - shift)) over unmasked entries
    outb = nc.dram_tensor("outbuf", [128, NSLICE + 1], f32, kind="ExternalOutput")

    from contextlib import ExitStack

    with ExitStack() as ctx:
        ec = ctx.enter_context
        c0 = ec(nc.sbuf_tensor("c0", [128, 2 + CH2], bf16))
        cbufs = [c0] + [
            ec(nc.sbuf_tensor(f"c{i}", [128, CH2], bf16)) for i in range(1, NCHUNK)
        ]
        mb = ec(nc.sbuf_tensor("mb", [128, NSLICE], f32))
        ns = ec(nc.sbuf_tensor("ns", [128, 1], f32))
        outsb = ec(nc.sbuf_tensor("outsb", [128, NSLICE + 1], f32))
        esums = ec(nc.sbuf_tensor("esums", [128, NCHUNK], f32))
        tmp = ec(nc.sbuf_tensor("tmp", [128, SPC], f32))
        ets = [
            ec(nc.sbuf_tensor(f"et{i}", [128, SPC], f32)) for i in range(NCHUNK)
        ]
        psA = ec(nc.psum_tensor("psA", [128, 512], f32))
        psB = ec(nc.psum_tensor("psB", [128, 512], f32))
        ch_sems = [ec(nc.semaphore(f"ch_sem{c}")) for c in range(NCHUNK)]
        out_sem = ec(nc.semaphore("out_sem"))
        mb_sem = ec(nc.semaphore("mb_sem"))
        ns_sem = ec(nc.semaphore("ns_sem"))
        pe_sem = ec(nc.semaphore("pe_sem"))
        dve_sem = ec(nc.semaphore("dve_sem"))
        act_sem = ec(nc.semaphore("act_sem"))

        def chunk_half(c, half):  # half 0 = hi, 1 = lo
            base = 2 if c == 0 else 0
            t = cbufs[c]
            return t[:, base + half * CCOLS : base + (half + 1) * CCOLS]

        with nc.Block() as block:

            @block.sync
            def _(sync):
                sync.dma_start(c0[:], candWHL[:, 0 : 2 + CH2]).then_inc(ch_sems[0], 16)
                for c in range(1, NCHUNK):
                    sync.dma_start(
                        cbufs[c][:], candWHL[:, 2 + c * CH2 : 2 + (c + 1) * CH2]
                    ).then_inc(ch_sems[c], 16)
                sync.wait_ge(dve_sem, NCHUNK + 1)  # all scores + rowsum in outsb
                sync.dma_start(outb[:], outsb[:]).then_inc(out_sem, 16)
                sync.wait_ge(out_sem, 16)

            @block.scalar
            def _(scalar):
                scalar.dma_start(mb[:], maskbias[:]).then_inc(mb_sem, 16)
                scalar.dma_start(ns[:], negshift[:]).then_inc(ns_sem, 16)
                scalar.wait_ge(ns_sem, 16)
                for c in range(NCHUNK):
                    scalar.wait_ge(dve_sem, c + 1)
                    scalar.activation(
                        ets[c][:],
                        outsb[:, c * SPC : (c + 1) * SPC],
                        Exp,
                        bias=ns[:],
                        scale=1.0,
                        accum_out=esums[:, c : c + 1],
                    ).then_inc(act_sem)

            @block.tensor
            def _(tensor):
                w_ap = c0[:, 0:2]
                for c in range(NCHUNK):
                    tensor.wait_ge(ch_sems[c], 16)
                    if c >= 2:
                        tensor.wait_ge(dve_sem, c - 1)  # psum bank reuse guard
                    ps = psA if c % 2 == 0 else psB
                    hi = chunk_half(c, 0)
                    lo = chunk_half(c, 1)
                    for si in range(SPC):
                        nc.tensor.matmul(
                            ps[:, 2 * si : 2 * si + 2],
                            hi[:, si * 128 : (si + 1) * 128],
                            w_ap,
                            start=True,
                            stop=False,
                        )
                        mm = nc.tensor.matmul(
                            ps[:, 2 * si : 2 * si + 2],
                            lo[:, si * 128 : (si + 1) * 128],
                            w_ap,
                            start=False,
                            stop=True,
                        )
                    mm.then_inc(pe_sem)

            @block.vector
            def _(vector):
                vector.wait_ge(mb_sem, 16)  # mb loaded
                for c in range(NCHUNK):
                    vector.wait_ge(pe_sem, c + 1)
                    ps = psA if c % 2 == 0 else psB
                    vector.tensor_add(
                        tmp[:],
                        ps[:, 0 : 2 * SPC : 2],
                        mb[:, c * SPC : (c + 1) * SPC],
                    )
                    vector.drain()
                    vector.tensor_add(
                        outsb[:, c * SPC : (c + 1) * SPC],
                        ps[:, 1 : 2 * SPC : 2],
                        tmp[:],
                    ).then_inc(dve_sem)
                    vector.drain()
                vector.wait_ge(act_sem, NCHUNK)
                vector.reduce_sum(
                    outsb[:, NSLICE : NSLICE + 1], esums[:], axis=X
                ).then_inc(dve_sem)

    nc.finalize()
    return nc


def get_nc():
    if "nc" not in _NC_CACHE:
        _NC_CACHE["nc"] = _build_nc()
    return _NC_CACHE["nc"]


def make_in_maps(cand, w, mask_np, shift):
    """Shard + lay out host inputs for the 8 cores."""
    import ml_dtypes

    bf16 = ml_dtypes.bfloat16

    wh = w.reshape(E).astype(bf16)
    wl = (w.reshape(E) - wh.astype(np.float32)).astype(bf16)
    wcol = np.stack([wh, wl], axis=1)  # [128, 2] bf16

    negshift = np.full((128, 1), -shift, np.float32)

    candT = cand.T  # [128, N] view
    in_maps = []
    for c in range(NCORES):
        xc = np.zeros((128, PAD), np.float32)
        xc[:, :SHARD] = candT[:, c * SHARD : (c + 1) * SHARD]
        hi = xc.astype(bf16)
        lo = (xc - hi.astype(np.float32)).astype(bf16)
        candWHL = np.empty((128, 2 + 2 * PAD), bf16)
        candWHL[:, 0:2] = wcol
        for k in range(NCHUNK):
            base = 2 + k * CH2
            candWHL[:, base : base + CCOLS] = hi[:, k * CCOLS : (k + 1) * CCOLS]
            candWHL[:, base + CCOLS : base + CH2] = lo[
                :, k * CCOLS : (k + 1) * CCOLS
            ]

        mflat = np.zeros(PAD, np.float32)
        mflat[:SHARD] = mask_np[0, c * SHARD : (c + 1) * SHARD]
        # layout [p, s] corresponds to local row s*128 + p
        mbias = np.ascontiguousarray(
            ((mflat - 1.0) * (-MASKVAL)).astype(np.float32).reshape(NSLICE, 128).T
        )
        in_maps.append(
            {"candWHL": candWHL, "maskbias": mbias, "negshift": negshift}
        )
    return in_maps


def kernel(current_node_emb, context_emb, candidate_node_embs, Wq, Wk, mask):
    global LAST_RESULTS, LAST_SCORES
    from concourse.bass_utils import run_bass_kernel_spmd

    cur = np.asarray(current_node_emb, np.float32)
    ctxe = np.asarray(context_emb, np.float32)
    cand = np.ascontiguousarray(np.asarray(candidate_node_embs, np.float32))
    Wq_np = np.asarray(Wq, np.float32)
    Wk_np = np.asarray(Wk, np.float32)
    mask_np = np.asarray(mask)

    # tiny query projection; scores = w @ cand.T with w = (combined @ Wq.T) @ Wk
    combined = np.concatenate([cur, ctxe], axis=1)  # [1, 2E]
    query = (combined @ Wq_np.T).astype(np.float32)  # [1, H]
    w = (query @ Wk_np).astype(np.float32)  # [1, E]

    # fixed exp shift: safe upper bound on any score
    shift = float(max(40.0, 16.0 * np.linalg.norm(w)))

    in_maps = make_in_maps(cand, w, mask_np, shift)
    nc = get_nc()
    res = run_bass_kernel_spmd(nc, in_maps, list(range(NCORES)))
    LAST_RESULTS = res

    # ---- gather / merge ----
    all_scores = np.empty(N_TOTAL, np.float32)
    rowsums = np.empty((NCORES, 128), np.float64)
    for c in range(NCORES):
        ob = np.asarray(res.results[c]["outbuf"])  # [128, 197]
        all_scores[c * SHARD : (c + 1) * SHARD] = ob[:, :NSLICE].T.reshape(-1)[:SHARD]
        rowsums[c] = ob[:, NSLICE]
    LAST_SCORES = all_scores

    # top-RESCORE candidates by device score; re-score them exactly on the
    # reference fp32 path (keys = cand @ Wk.T, s = query @ keys.T)
    sel = np.argpartition(all_scores, N_TOTAL - RESCORE)[N_TOTAL - RESCORE :]
    keys_sel = (cand[sel] @ Wk_np.T).astype(np.float32)  # [R, H]
    s_sel = (query @ keys_sel.T).astype(np.float32)[0]  # [R]

    # merge softmax statistics (the "all-reduce" step, done at gather time)
    m = np.float32(s_sel.max())
    Z = np.float32(np.exp(np.float64(shift) - np.float64(m)) * rowsums.sum())

    # exact probabilities of the re-scored candidates; top-50 threshold in
    # probability space, exactly like the reference
    p_sel = (np.exp(s_sel - m) / Z).astype(np.float32)
    th = np.sort(p_sel)[-TOPK]
    keep = p_sel >= th
    p_top = p_sel * keep
    S = p_top.sum(dtype=np.float32)
    fil_top = (p_top / (S + np.float32(1e-10))).astype(np.float32)

    log_probs_all = np.full((1, N_TOTAL), np.log(np.float32(1e-10)), np.float32)
    logits = np.full((1, N_TOTAL), -np.inf, np.float32)
    sel_keep = sel[keep]
    fil_keep = fil_top[keep]
    log_probs_all[0, sel_keep] = np.log(fil_keep + np.float32(1e-10))
    logits[0, sel_keep] = np.log(fil_keep)

    # categorical sample with jax key 42 (on host CPU, exact reference RNG)
    import jax

    cpu = jax.devices("cpu")[0]
    with jax.default_device(cpu):
        action_idx = np.asarray(
            jax.random.categorical(
                jax.random.key(42), jax.numpy.asarray(logits), axis=1
            )
        )
    log_prob_action = np.take_along_axis(logits, action_idx[:, None], axis=1)[:, 0]

    return log_probs_all, log_prob_action, action_idx


# revision 25
# speedup vs baseline: 2.1558x; 1.0371x over previous
"""AttentionDecoder (topk_masking) Trainium2 kernel.

Algorithm (matches the jax reference):
  combined = cat([current, context])           # [1, 2E]
  query    = combined @ Wq.T                   # [1, H]
  scores   = query @ Wk @ cand.T               # [1, N]  (keys folded into w)
  masked softmax -> top-50 filter -> renormalize -> log probs + categorical
  sample (jax key 42).

Distribution: candidates are sharded row-wise over 8 NeuronCores (25000
rows/core, zero-padded to 25088 = 196*128) and shipped pre-transposed in
a compressed split format: c = hi + lo with hi = bf16(c) (2 bytes) and
lo = fp8_e4m3(512 * (c - hi)) (1 byte) — 3 bytes/element instead of 4,
cutting the HBM stream by 25% while keeping ~1e-3 absolute score
accuracy.  Per 128-candidate slice the PE runs two matmuls (candidate
tiles stationary): bf16 hi @ [w_hi, w_lo] and fp8 lo @ [w8, 0]; DVE
combines psum columns as hi0 + hi1 + lo/512 + maskbias.  The partial
softmax statistics sum(exp(s - shift)) (fixed shift) are computed per
chunk on ACT, all overlapped with the next chunk's DMA+matmuls.

The kernel is written in raw Bass (hand-placed semaphores, no Tile
scheduler) so the only fixed overhead is the NRT preamble — the Tile
exit-barrier butterfly (~10 us) is avoided.  The w vectors ride in the
first bytes of the big candidate tensor so the PE can start as soon as
chunk 0 lands.

The host gathers the per-core scores + softmax partials, merges the
statistics (the "all-reduce" step), selects the top-100 by device
score, re-scores exactly those candidates on the reference fp32 path
(100 rows — negligible), and assembles the outputs; the softmax
normalizer cancels in the renormalized top-50 distribution, so the
outputs are fp32-exact and robust to the device-side compression.

The kernel is memory-bound on the HBM read of the 76.8 MB of compressed
candidate embeddings (9.6 MB/core).
"""

import os

import numpy as np

E = 128
N_TOTAL = 200000
NCORES = 8
SHARD = N_TOTAL // NCORES       # 25000 rows per core
NSLICE = 196                    # 128-wide score slices per core
PAD = NSLICE * 128              # 25088 padded rows per core
NCHUNK = 7
SPC = NSLICE // NCHUNK          # 28 slices per chunk
CCOLS = SPC * 128               # 3584 candidate columns per chunk
HIB = 2 * CCOLS                 # 7168 bytes of hi per chunk
CHB = 3 * CCOLS                 # 10752 bytes per chunk (hi + lo)
PREFIX = 8                      # [wh,wl]bf16 (4B) + [w8,0]fp8 (2B) + pad (2B)
TOTB = PREFIX + NCHUNK * CHB    # 75272 bytes per partition
LOSCALE = 512.0                 # lo stored as fp8(512 * residual)
MASKVAL = np.float32(-1.0e9)    # additive mask bias (exp underflows to 0)
TOPK = 50
RESCORE = 100                   # candidates re-scored exactly on host

_NC_CACHE = {}
LAST_RESULTS = None  # BassKernelResults of the most recent run (for profiling)
LAST_SCORES = None  # gathered masked scores of the most recent run (diagnostics)


def _build_nc():
    """Raw-Bass per-core program (identical on all 8 cores)."""
    import concourse.bacc as bacc
    from concourse import mybir

    f32 = mybir.dt.float32
    bf16 = mybir.dt.bfloat16
    f8 = mybir.dt.float8e4
    u8 = mybir.dt.uint8
    X = mybir.AxisListType.X
    Exp = mybir.ActivationFunctionType.Exp
    Alu = mybir.AluOpType

    nc = bacc.Bacc(
        "TRN2",
        target_bir_lowering=False,
        debug=False,
        enable_asserts=False,
        num_devices=NCORES,
    )

    candB = nc.dram_tensor("candB", [128, TOTB], u8, kind="ExternalInput")
    maskbias = nc.dram_tensor("maskbias", [128, NSLICE], f32, kind="ExternalInput")
    negshift = nc.dram_tensor("negshift", [128, 1], f32, kind="ExternalInput")
    # outbuf[:, :196] = masked scores, [:, 196] = per-partition
    # sum(exp(s - shift)) over unmasked entries
    outb = nc.dram_tensor("outbuf", [128, NSLICE + 1], f32, kind="ExternalOutput")

    from contextlib import ExitStack

    with ExitStack() as ctx:
        ec = ctx.enter_context
        c0 = ec(nc.sbuf_tensor("c0", [128, PREFIX + CHB], u8))
        cbufs = [c0] + [
            ec(nc.sbuf_tensor(f"c{i}", [128, CHB], u8)) for i in range(1, NCHUNK)
        ]
        mb = ec(nc.sbuf_tensor("mb", [128, NSLICE], f32))
        ns = ec(nc.sbuf_tensor("ns", [128, 1], f32))
        outsb = ec(nc.sbuf_tensor("outsb", [128, NSLICE + 1], f32))
        esums = ec(nc.sbuf_tensor("esums", [128, NCHUNK], f32))
        tmp = ec(nc.sbuf_tensor("tmp", [128, SPC], f32))
        tmp2 = ec(nc.sbuf_tensor("tmp2", [128, SPC], f32))
        ets = [
            ec(nc.sbuf_tensor(f"et{i}", [128, SPC], f32)) for i in range(NCHUNK)
        ]
        psH = [
            ec(nc.psum_tensor("psHA", [128, 512], f32)),
            ec(nc.psum_tensor("psHB", [128, 512], f32)),
        ]
        psL = [
            ec(nc.psum_tensor("psLA", [128, 512], f32)),
            ec(nc.psum_tensor("psLB", [128, 512], f32)),
        ]
        ch_sems = [ec(nc.semaphore(f"ch_sem{c}")) for c in range(NCHUNK)]
        out_sem = ec(nc.semaphore("out_sem"))
        mb_sem = ec(nc.semaphore("mb_sem"))
        ns_sem = ec(nc.semaphore("ns_sem"))
        pe_sem = ec(nc.semaphore("pe_sem"))
        dve_sem = ec(nc.semaphore("dve_sem"))
        act_sem = ec(nc.semaphore("act_sem"))

        def chunk_aps(c):
            t = cbufs[c]
            base = PREFIX if c == 0 else 0
            hi = t[:, base : base + HIB].bitcast(bf16)  # [128, 3584]
            lo = t[:, base + HIB : base + CHB].bitcast(f8)  # [128, 3584]
            return hi, lo

        with nc.Block() as block:

            @block.sync
            def _(sync):
                sync.dma_start(c0[:], candB[:, 0 : PREFIX + CHB]).then_inc(
                    ch_sems[0], 16
                )
                for c in range(1, NCHUNK):
                    sync.dma_start(
                        cbufs[c][:],
                        candB[:, PREFIX + c * CHB : PREFIX + (c + 1) * CHB],
                    ).then_inc(ch_sems[c], 16)
                sync.wait_ge(dve_sem, NCHUNK + 1)  # all scores + rowsum in outsb
                sync.dma_start(outb[:], outsb[:]).then_inc(out_sem, 16)
                sync.wait_ge(out_sem, 16)

            @block.scalar
            def _(scalar):
                scalar.dma_start(mb[:], maskbias[:]).then_inc(mb_sem, 16)
                scalar.dma_start(ns[:], negshift[:]).then_inc(ns_sem, 16)
                scalar.wait_ge(ns_sem, 16)
                for c in range(NCHUNK):
                    scalar.wait_ge(dve_sem, c + 1)
                    scalar.activation(
                        ets[c][:],
                        outsb[:, c * SPC : (c + 1) * SPC],
                        Exp,
                        bias=ns[:],
                        scale=1.0,
                        accum_out=esums[:, c : c + 1],
                    ).then_inc(act_sem)

            @block.tensor
            def _(tensor):
                rhs_hi = c0[:, 0:4].bitcast(bf16)  # [128, 2] = [wh, wl]
                rhs_lo = c0[:, 4:6].bitcast(f8)  # [128, 2] = [w8, 0]
                for c in range(NCHUNK):
                    tensor.wait_ge(ch_sems[c], 16)
                    if c >= 2:
                        tensor.wait_ge(dve_sem, c - 1)  # psum bank reuse guard
                    pH = psH[c % 2]
                    pL = psL[c % 2]
                    hi, lo = chunk_aps(c)
                    for si in range(SPC):
                        nc.tensor.matmul(
                            pH[:, 2 * si : 2 * si + 2],
                            hi[:, si * 128 : (si + 1) * 128],
                            rhs_hi,
                            start=True,
                            stop=True,
                        )
                        mm = nc.tensor.matmul(
                            pL[:, 2 * si : 2 * si + 2],
                            lo[:, si * 128 : (si + 1) * 128],
                            rhs_lo,
                            start=True,
                            stop=True,
                        )
                    mm.then_inc(pe_sem)

            @block.vector
            def _(vector):
                vector.wait_ge(mb_sem, 16)  # mb loaded
                for c in range(NCHUNK):
                    vector.wait_ge(pe_sem, c + 1)
                    pH = psH[c % 2]
                    pL = psL[c % 2]
                    # scores = hi0 + hi1 + lo0/512 + maskbias
                    vector.tensor_add(
                        tmp[:],
                        pH[:, 0 : 2 * SPC : 2],
                        mb[:, c * SPC : (c + 1) * SPC],
                    )
                    vector.drain()
                    vector.tensor_add(tmp2[:], pH[:, 1 : 2 * SPC : 2], tmp[:])
                    vector.drain()
                    vector.scalar_tensor_tensor(
                        outsb[:, c * SPC : (c + 1) * SPC],
                        pL[:, 0 : 2 * SPC : 2],
                        1.0 / LOSCALE,
                        tmp2[:],
                        op0=Alu.mult,
                        op1=Alu.add,
                    ).then_inc(dve_sem)
                    vector.drain()
                vector.wait_ge(act_sem, NCHUNK)
                vector.reduce_sum(
                    outsb[:, NSLICE : NSLICE + 1], esums[:], axis=X
                ).then_inc(dve_sem)

    nc.finalize()
    return nc


def get_nc():
    if "nc" not in _NC_CACHE:
        _NC_CACHE["nc"] = _build_nc()
    return _NC_CACHE["nc"]


def make_in_maps(cand, w, mask_np, shift):
    """Shard + lay out host inputs for the 8 cores."""
    import ml_dtypes

    bf16 = ml_dtypes.bfloat16
    f8 = ml_dtypes.float8_e4m3

    wf = w.reshape(E)
    wh = wf.astype(bf16)
    wl = (wf - wh.astype(np.float32)).astype(bf16)
    w_bf = np.stack([wh, wl], axis=1)  # [128, 2] bf16
    w_f8 = np.zeros((E, 2), f8)
    w_f8[:, 0] = wf.astype(f8)

    prefix = np.zeros((128, PREFIX), np.uint8)
    prefix[:, 0:4] = np.ascontiguousarray(w_bf).view(np.uint8)
    prefix[:, 4:6] = np.ascontiguousarray(w_f8).view(np.uint8)

    negshift = np.full((128, 1), -shift, np.float32)

    candT = cand.T  # [128, N] view
    in_maps = []
    for c in range(NCORES):
        xc = np.zeros((128, PAD), np.float32)
        xc[:, :SHARD] = candT[:, c * SHARD : (c + 1) * SHARD]
        hi = xc.astype(bf16)
        lo = ((xc - hi.astype(np.float32)) * LOSCALE).astype(f8)
        hi_u8 = hi.view(np.uint8)  # [128, 2*PAD]
        lo_u8 = lo.view(np.uint8)  # [128, PAD]
        candB = np.empty((128, TOTB), np.uint8)
        candB[:, 0:PREFIX] = prefix
        for k in range(NCHUNK):
            base = PREFIX + k * CHB
            candB[:, base : base + HIB] = hi_u8[:, k * HIB : (k + 1) * HIB]
            candB[:, base + HIB : base + CHB] = lo_u8[
                :, k * CCOLS : (k + 1) * CCOLS
            ]

        mflat = np.zeros(PAD, np.float32)
        mflat[:SHARD] = mask_np[0, c * SHARD : (c + 1) * SHARD]
        # layout [p, s] corresponds to local row s*128 + p
        mbias = np.ascontiguousarray(
            ((mflat - 1.0) * (-MASKVAL)).astype(np.float32).reshape(NSLICE, 128).T
        )
        in_maps.append({"candB": candB, "maskbias": mbias, "negshift": negshift})
    return in_maps


def kernel(current_node_emb, context_emb, candidate_node_embs, Wq, Wk, mask):
    global LAST_RESULTS, LAST_SCORES
    from concourse.bass_utils import run_bass_kernel_spmd

    cur = np.asarray(current_node_emb, np.float32)
    ctxe = np.asarray(context_emb, np.float32)
    cand = np.ascontiguousarray(np.asarray(candidate_node_embs, np.float32))
    Wq_np = np.asarray(Wq, np.float32)
    Wk_np = np.asarray(Wk, np.float32)
    mask_np = np.asarray(mask)

    # tiny query projection; scores = w @ cand.T with w = (combined @ Wq.T) @ Wk
    combined = np.concatenate([cur, ctxe], axis=1)  # [1, 2E]
    query = (combined @ Wq_np.T).astype(np.float32)  # [1, H]
    w = (query @ Wk_np).astype(np.float32)  # [1, E]

    # fixed exp shift: safe upper bound on any score
    shift = float(max(40.0, 16.0 * np.linalg.norm(w)))

    in_maps = make_in_maps(cand, w, mask_np, shift)
    nc = get_nc()
    res = run_bass_kernel_spmd(nc, in_maps, list(range(NCORES)))
    LAST_RESULTS = res

    # ---- gather / merge ----
    all_scores = np.empty(N_TOTAL, np.float32)
    rowsums = np.empty((NCORES, 128), np.float64)
    for c in range(NCORES):
        ob = np.asarray(res.results[c]["outbuf"])  # [128, 197]
        all_scores[c * SHARD : (c + 1) * SHARD] = ob[:, :NSLICE].T.reshape(-1)[:SHARD]
        rowsums[c] = ob[:, NSLICE]
    LAST_SCORES = all_scores

    # top-RESCORE candidates by device score; re-score them exactly on the
    # reference fp32 path (keys = cand @ Wk.T, s = query @ keys.T)
    sel = np.argpartition(all_scores, N_TOTAL - RESCORE)[N_TOTAL - RESCORE :]
    keys_sel = (cand[sel] @ Wk_np.T).astype(np.float32)  # [R, H]
    s_sel = (query @ keys_sel.T).astype(np.float32)[0]  # [R]

    # merge softmax statistics (the "all-reduce" step, done at gather time)
    m = np.float32(s_sel.max())
    Z = np.float32(np.exp(np.float64(shift) - np.float64(m)) * rowsums.sum())

    # exact probabilities of the re-scored candidates; top-50 threshold in
    # probability space, exactly like the reference
    p_sel = (np.exp(s_sel - m) / Z).astype(np.float32)
    th = np.sort(p_sel)[-TOPK]
    keep = p_sel >= th
    p_top = p_sel * keep
    S = p_top.sum(dtype=np.float32)
    fil_top = (p_top / (S + np.float32(1e-10))).astype(np.float32)

    log_probs_all = np.full((1, N_TOTAL), np.log(np.float32(1e-10)), np.float32)
    logits = np.full((1, N_TOTAL), -np.inf, np.float32)
    sel_keep = sel[keep]
    fil_keep = fil_top[keep]
    log_probs_all[0, sel_keep] = np.log(fil_keep + np.float32(1e-10))
    logits[0, sel_keep] = np.log(fil_keep)

    # categorical sample with jax key 42 (on host CPU, exact reference RNG)
    import jax

    cpu = jax.devices("cpu")[0]
    with jax.default_device(cpu):
        action_idx = np.asarray(
            jax.random.categorical(
                jax.random.key(42), jax.numpy.asarray(logits), axis=1
            )
        )
    log_prob_action = np.take_along_axis(logits, action_idx[:, None], axis=1)[:, 0]

    return log_probs_all, log_prob_action, action_idx


# revision 26
# speedup vs baseline: 2.8145x; 1.3056x over previous
"""AttentionDecoder (topk_masking) Trainium2 kernel.

Algorithm (matches the jax reference):
  combined = cat([current, context])           # [1, 2E]
  query    = combined @ Wq.T                   # [1, H]
  scores   = query @ Wk @ cand.T               # [1, N]  (keys folded into w)
  masked softmax -> top-50 filter -> renormalize -> log probs + categorical
  sample (jax key 42).

Distribution: candidates are sharded row-wise over 8 NeuronCores (25000
rows/core, zero-padded to 25088 = 196*128) and shipped pre-transposed,
compressed to bf16 (2 bytes/element — half the HBM traffic of fp32).
Per 128-candidate slice one PE matmul (candidate tile stationary,
rhs = [w_hi, w_lo] split of the query vector) produces the scores in
fp32 PSUM with ~6e-3 absolute accuracy; masking (additive -1e9) and the
partial softmax statistics sum(exp(s - shift)) (fixed shift) follow per
chunk on DVE/ACT, overlapped with the next chunk's DMA+matmuls.

The bf16 rounding is fully healed on the host: it gathers the per-core
scores + softmax partials, merges the statistics (the "all-reduce"
step), selects the top-100 by device score — the true top-50 is inside
with >25 sigma of margin against the ~0.3 score gap at rank 100 — and
re-scores exactly those 100 candidates on the reference fp32 path.  The
softmax normalizer cancels in the renormalized top-50 distribution, so
the outputs are fp32-exact.

The kernel is written in raw Bass (hand-placed semaphores, no Tile
scheduler) so the only fixed overhead is the NRT preamble — the Tile
exit-barrier butterfly (~10 us) is avoided.  The w vector rides in the
first bytes of the big candidate tensor so the PE can start as soon as
chunk 0 lands.

The kernel is memory-bound on the HBM read of the 51.2 MB of compressed
candidate embeddings (6.4 MB/core, HBM shared per core pair).
"""

import os

import numpy as np

E = 128
N_TOTAL = 200000
NCORES = 8
SHARD = N_TOTAL // NCORES       # 25000 rows per core
NSLICE = 196                    # 128-wide score slices per core
PAD = NSLICE * 128              # 25088 padded rows per core
NCHUNK = 4
SPC = NSLICE // NCHUNK          # 49 slices per chunk
CCOLS = SPC * 128               # 6272 candidate columns per chunk
CHB = 2 * CCOLS                 # 12544 bytes per chunk (bf16)
PREFIX = 4                      # [wh, wl] bf16
TOTB = PREFIX + NCHUNK * CHB    # 50180 bytes per partition
MASKVAL = np.float32(-1.0e9)    # additive mask bias (exp underflows to 0)
TOPK = 50
RESCORE = 100                   # candidates re-scored exactly on host

_NC_CACHE = {}
LAST_RESULTS = None  # BassKernelResults of the most recent run (for profiling)
LAST_SCORES = None  # gathered masked scores of the most recent run (diagnostics)


def _build_nc():
    """Raw-Bass per-core program (identical on all 8 cores)."""
    import concourse.bacc as bacc
    from concourse import mybir

    f32 = mybir.dt.float32
    bf16 = mybir.dt.bfloat16
    u8 = mybir.dt.uint8
    X = mybir.AxisListType.X
    Exp = mybir.ActivationFunctionType.Exp

    nc = bacc.Bacc(
        "TRN2",
        target_bir_lowering=False,
        debug=False,
        enable_asserts=False,
        num_devices=NCORES,
    )

    candB = nc.dram_tensor("candB", [128, TOTB], u8, kind="ExternalInput")
    maskbias = nc.dram_tensor("maskbias", [128, NSLICE], f32, kind="ExternalInput")
    negshift = nc.dram_tensor("negshift", [128, 1], f32, kind="ExternalInput")
    # outbuf[:, :196] = masked scores, [:, 196] = per-partition
    # sum(exp(s - shift)) over unmasked entries
    outb = nc.dram_tensor("outbuf", [128, NSLICE + 1], f32, kind="ExternalOutput")

    from contextlib import ExitStack

    with ExitStack() as ctx:
        ec = ctx.enter_context
        c0 = ec(nc.sbuf_tensor("c0", [128, PREFIX + CHB], u8))
        cbufs = [c0] + [
            ec(nc.sbuf_tensor(f"c{i}", [128, CHB], u8)) for i in range(1, NCHUNK)
        ]
        mb = ec(nc.sbuf_tensor("mb", [128, NSLICE], f32))
        ns = ec(nc.sbuf_tensor("ns", [128, 1], f32))
        outsb = ec(nc.sbuf_tensor("outsb", [128, NSLICE + 1], f32))
        esums = ec(nc.sbuf_tensor("esums", [128, NCHUNK], f32))
        tmp = ec(nc.sbuf_tensor("tmp", [128, SPC], f32))
        ets = [
            ec(nc.sbuf_tensor(f"et{i}", [128, SPC], f32)) for i in range(NCHUNK)
        ]
        psH = [
            ec(nc.psum_tensor("psHA", [128, 512], f32)),
            ec(nc.psum_tensor("psHB", [128, 512], f32)),
        ]
        ch_sems = [ec(nc.semaphore(f"ch_sem{c}")) for c in range(NCHUNK)]
        out_sem = ec(nc.semaphore("out_sem"))
        mb_sem = ec(nc.semaphore("mb_sem"))
        ns_sem = ec(nc.semaphore("ns_sem"))
        pe_sem = ec(nc.semaphore("pe_sem"))
        dve_sem = ec(nc.semaphore("dve_sem"))
        act_sem = ec(nc.semaphore("act_sem"))

        def chunk_hi(c):
            t = cbufs[c]
            base = PREFIX if c == 0 else 0
            return t[:, base : base + CHB].bitcast(bf16)  # [128, 6272]

        with nc.Block() as block:

            @block.sync
            def _(sync):
                sync.dma_start(c0[:], candB[:, 0 : PREFIX + CHB]).then_inc(
                    ch_sems[0], 16
                )
                for c in range(1, NCHUNK):
                    sync.dma_start(
                        cbufs[c][:],
                        candB[:, PREFIX + c * CHB : PREFIX + (c + 1) * CHB],
                    ).then_inc(ch_sems[c], 16)
                sync.wait_ge(dve_sem, NCHUNK + 1)  # all scores + rowsum in outsb
                sync.dma_start(outb[:], outsb[:]).then_inc(out_sem, 16)
                sync.wait_ge(out_sem, 16)

            @block.scalar
            def _(scalar):
                scalar.dma_start(mb[:], maskbias[:]).then_inc(mb_sem, 16)
                scalar.dma_start(ns[:], negshift[:]).then_inc(ns_sem, 16)
                scalar.wait_ge(ns_sem, 16)
                for c in range(NCHUNK):
                    scalar.wait_ge(dve_sem, c + 1)
                    scalar.activation(
                        ets[c][:],
                        outsb[:, c * SPC : (c + 1) * SPC],
                        Exp,
                        bias=ns[:],
                        scale=1.0,
                        accum_out=esums[:, c : c + 1],
                    ).then_inc(act_sem)

            @block.tensor
            def _(tensor):
                rhs_hi = c0[:, 0:4].bitcast(bf16)  # [128, 2] = [wh, wl]
                for c in range(NCHUNK):
                    tensor.wait_ge(ch_sems[c], 16)
                    if c >= 2:
                        tensor.wait_ge(dve_sem, c - 1)  # psum bank reuse guard
                    pH = psH[c % 2]
                    hi = chunk_hi(c)
                    for si in range(SPC):
                        mm = nc.tensor.matmul(
                            pH[:, 2 * si : 2 * si + 2],
                            hi[:, si * 128 : (si + 1) * 128],
                            rhs_hi,
                            start=True,
                            stop=True,
                        )
                    mm.then_inc(pe_sem)

            @block.vector
            def _(vector):
                vector.wait_ge(mb_sem, 16)  # mb loaded
                for c in range(NCHUNK):
                    vector.wait_ge(pe_sem, c + 1)
                    pH = psH[c % 2]
                    # scores = hi0 + hi1 + maskbias
                    vector.tensor_add(
                        tmp[:],
                        pH[:, 0 : 2 * SPC : 2],
                        mb[:, c * SPC : (c + 1) * SPC],
                    )
                    vector.drain()
                    vector.tensor_add(
                        outsb[:, c * SPC : (c + 1) * SPC],
                        pH[:, 1 : 2 * SPC : 2],
                        tmp[:],
                    ).then_inc(dve_sem)
                    vector.drain()
                vector.wait_ge(act_sem, NCHUNK)
                vector.reduce_sum(
                    outsb[:, NSLICE : NSLICE + 1], esums[:], axis=X
                ).then_inc(dve_sem)

    nc.finalize()
    return nc


def get_nc():
    if "nc" not in _NC_CACHE:
        _NC_CACHE["nc"] = _build_nc()
    return _NC_CACHE["nc"]


def make_in_maps(cand, w, mask_np, shift):
    """Shard + lay out host inputs for the 8 cores."""
    import ml_dtypes

    bf16 = ml_dtypes.bfloat16

    wf = w.reshape(E)
    wh = wf.astype(bf16)
    wl = (wf - wh.astype(np.float32)).astype(bf16)
    w_bf = np.stack([wh, wl], axis=1)  # [128, 2] bf16

    prefix = np.ascontiguousarray(w_bf).view(np.uint8)  # [128, 4]

    negshift = np.full((128, 1), -shift, np.float32)

    candT = cand.T  # [128, N] view
    in_maps = []
    for c in range(NCORES):
        xc = np.zeros((128, PAD), np.float32)
        xc[:, :SHARD] = candT[:, c * SHARD : (c + 1) * SHARD]
        hi_u8 = xc.astype(bf16).view(np.uint8)  # [128, 2*PAD]
        candB = np.empty((128, TOTB), np.uint8)
        candB[:, 0:PREFIX] = prefix
        candB[:, PREFIX:] = hi_u8

        mflat = np.zeros(PAD, np.float32)
        mflat[:SHARD] = mask_np[0, c * SHARD : (c + 1) * SHARD]
        # layout [p, s] corresponds to local row s*128 + p
        mbias = np.ascontiguousarray(
            ((mflat - 1.0) * (-MASKVAL)).astype(np.float32).reshape(NSLICE, 128).T
        )
        in_maps.append({"candB": candB, "maskbias": mbias, "negshift": negshift})
    return in_maps


def kernel(current_node_emb, context_emb, candidate_node_embs, Wq, Wk, mask):
    global LAST_RESULTS, LAST_SCORES
    from concourse.bass_utils import run_bass_kernel_spmd

    cur = np.asarray(current_node_emb, np.float32)
    ctxe = np.asarray(context_emb, np.float32)
    cand = np.ascontiguousarray(np.asarray(candidate_node_embs, np.float32))
    Wq_np = np.asarray(Wq, np.float32)
    Wk_np = np.asarray(Wk, np.float32)
    mask_np = np.asarray(mask)

    # tiny query projection; scores = w @ cand.T with w = (combined @ Wq.T) @ Wk
    combined = np.concatenate([cur, ctxe], axis=1)  # [1, 2E]
    query = (combined @ Wq_np.T).astype(np.float32)  # [1, H]
    w = (query @ Wk_np).astype(np.float32)  # [1, E]

    # fixed exp shift: safe upper bound on any score
    shift = float(max(40.0, 16.0 * np.linalg.norm(w)))

    in_maps = make_in_maps(cand, w, mask_np, shift)
    nc = get_nc()
    res = run_bass_kernel_spmd(nc, in_maps, list(range(NCORES)))
    LAST_RESULTS = res

    # ---- gather / merge ----
    all_scores = np.empty(N_TOTAL, np.float32)
    rowsums = np.empty((NCORES, 128), np.float64)
    for c in range(NCORES):
        ob = np.asarray(res.results[c]["outbuf"])  # [128, 197]
        all_scores[c * SHARD : (c + 1) * SHARD] = ob[:, :NSLICE].T.reshape(-1)[:SHARD]
        rowsums[c] = ob[:, NSLICE]
    LAST_SCORES = all_scores

    # top-RESCORE candidates by device score; re-score them exactly on the
    # reference fp32 path (keys = cand @ Wk.T, s = query @ keys.T)
    sel = np.argpartition(all_scores, N_TOTAL - RESCORE)[N_TOTAL - RESCORE :]
    keys_sel = (cand[sel] @ Wk_np.T).astype(np.float32)  # [R, H]
    s_sel = (query @ keys_sel.T).astype(np.float32)[0]  # [R]

    # merge softmax statistics (the "all-reduce" step, done at gather time)
    m = np.float32(s_sel.max())
    Z = np.float32(np.exp(np.float64(shift) - np.float64(m)) * rowsums.sum())

    # exact probabilities of the re-scored candidates; top-50 threshold in
    # probability space, exactly like the reference
    p_sel = (np.exp(s_sel - m) / Z).astype(np.float32)
    th = np.sort(p_sel)[-TOPK]
    keep = p_sel >= th
    p_top = p_sel * keep
    S = p_top.sum(dtype=np.float32)
    fil_top = (p_top / (S + np.float32(1e-10))).astype(np.float32)

    log_probs_all = np.full((1, N_TOTAL), np.log(np.float32(1e-10)), np.float32)
    logits = np.full((1, N_TOTAL), -np.inf, np.float32)
    sel_keep = sel[keep]
    fil_keep = fil_top[keep]
    log_probs_all[0, sel_keep] = np.log(fil_keep + np.float32(1e-10))
    logits[0, sel_keep] = np.log(fil_keep)

    # categorical sample with jax key 42 (on host CPU, exact reference RNG)
    import jax

    cpu = jax.devices("cpu")[0]
    with jax.default_device(cpu):
        action_idx = np.asarray(
            jax.random.categorical(
                jax.random.key(42), jax.numpy.asarray(logits), axis=1
            )
        )
    log_prob_action = np.take_along_axis(logits, action_idx[:, None], axis=1)[:, 0]

    return log_probs_all, log_prob_action, action_idx


# revision 29
# speedup vs baseline: 2.9860x; 1.0609x over previous
"""AttentionDecoder (topk_masking) Trainium2 kernel.

Algorithm (matches the jax reference):
  combined = cat([current, context])           # [1, 2E]
  query    = combined @ Wq.T                   # [1, H]
  scores   = query @ Wk @ cand.T               # [1, N]  (keys folded into w)
  masked softmax -> top-50 filter -> renormalize -> log probs + categorical
  sample (jax key 42).

Distribution: candidates are sharded row-wise over 8 NeuronCores (25000
rows/core, zero-padded to 25088 = 196*128) and shipped pre-transposed,
compressed to bf16 (2 bytes/element — half the HBM traffic of fp32).
Per 128-candidate slice one PE matmul (candidate tile stationary,
rhs = [w_hi, w_lo] split of the query vector) produces the scores in
fp32 PSUM with ~6e-3 absolute accuracy; masking (additive -1e9) and the
partial softmax statistics sum(exp(s - shift)) (fixed shift) follow per
chunk on DVE/ACT, overlapped with the next chunk's DMA+matmuls.

The bf16 rounding is fully healed on the host: it gathers the per-core
scores + softmax partials, merges the statistics (the "all-reduce"
step), selects the top-100 by device score — the true top-50 is inside
with >25 sigma of margin against the ~0.3 score gap at rank 100 — and
re-scores exactly those 100 candidates on the reference fp32 path.  The
softmax normalizer cancels in the renormalized top-50 distribution, so
the outputs are fp32-exact.

The kernel is written in raw Bass (hand-placed semaphores, no Tile
scheduler) so the only fixed overhead is the NRT preamble — the Tile
exit-barrier butterfly (~10 us) is avoided.  The w vector rides in the
first bytes of the big candidate tensor so the PE can start as soon as
chunk 0 lands.

The kernel is memory-bound on the HBM read of the 51.2 MB of compressed
candidate embeddings (6.4 MB/core, HBM shared per core pair).
"""

import os

import numpy as np

E = 128
N_TOTAL = 200000
NCORES = 8
SHARD = N_TOTAL // NCORES       # 25000 rows per core
NSLICE = 196                    # 128-wide score slices per core
PAD = NSLICE * 128              # 25088 padded rows per core
NCHUNK = 4
SPCS = [56, 56, 56, 28]         # slices per chunk (small last chunk = short tail)
SOFF = [0, 56, 112, 168]        # slice offset of each chunk
CHBS = [s * 256 for s in SPCS]  # bytes per chunk (bf16: 128 cols * 2B per slice)
PREFIX = 4                      # [wh, wl] bf16
TOTB = PREFIX + 2 * PAD         # 50180 bytes per partition
MASKVAL = np.float32(-1.0e9)    # additive mask bias (exp underflows to 0)
TOPK = 50
RESCORE = 100                   # candidates re-scored exactly on host

_NC_CACHE = {}
LAST_RESULTS = None  # BassKernelResults of the most recent run (for profiling)
LAST_SCORES = None  # gathered masked scores of the most recent run (diagnostics)


def _build_nc():
    """Raw-Bass per-core program (identical on all 8 cores)."""
    import concourse.bacc as bacc
    from concourse import mybir

    f32 = mybir.dt.float32
    bf16 = mybir.dt.bfloat16
    u8 = mybir.dt.uint8
    X = mybir.AxisListType.X
    Exp = mybir.ActivationFunctionType.Exp

    nc = bacc.Bacc(
        "TRN2",
        target_bir_lowering=False,
        debug=False,
        enable_asserts=False,
        num_devices=NCORES,
    )

    candB = nc.dram_tensor("candB", [128, TOTB], u8, kind="ExternalInput")
    maskbias = nc.dram_tensor("maskbias", [128, NSLICE], f32, kind="ExternalInput")
    negshift = nc.dram_tensor("negshift", [128, 1], f32, kind="ExternalInput")
    # outbuf[:, :196] = masked scores, [:, 196] = per-partition
    # sum(exp(s - shift)) over unmasked entries
    outb = nc.dram_tensor("outbuf", [128, NSLICE + 1], f32, kind="ExternalOutput")

    from contextlib import ExitStack

    with ExitStack() as ctx:
        ec = ctx.enter_context
        c0 = ec(nc.sbuf_tensor("c0", [128, PREFIX + CHBS[0]], u8))
        cbufs = [c0] + [
            ec(nc.sbuf_tensor(f"c{i}", [128, CHBS[i]], u8))
            for i in range(1, NCHUNK)
        ]
        mb = ec(nc.sbuf_tensor("mb", [128, NSLICE], f32))
        ns = ec(nc.sbuf_tensor("ns", [128, 1], f32))
        outsb = ec(nc.sbuf_tensor("outsb", [128, NSLICE + 1], f32))
        esums = ec(nc.sbuf_tensor("esums", [128, NCHUNK], f32))
        tmp = ec(nc.sbuf_tensor("tmp", [128, max(SPCS)], f32))
        ets = [
            ec(nc.sbuf_tensor(f"et{i}", [128, SPCS[i]], f32))
            for i in range(NCHUNK)
        ]
        psH = [
            ec(nc.psum_tensor("psHA", [128, 512], f32)),
            ec(nc.psum_tensor("psHB", [128, 512], f32)),
        ]
        ch_sems = [ec(nc.semaphore(f"ch_sem{c}")) for c in range(NCHUNK)]
        out_sem = ec(nc.semaphore("out_sem"))
        out2_sem = ec(nc.semaphore("out2_sem"))
        mb_sem = ec(nc.semaphore("mb_sem"))
        ns_sem = ec(nc.semaphore("ns_sem"))
        pe_sem = ec(nc.semaphore("pe_sem"))
        dve_sem = ec(nc.semaphore("dve_sem"))
        act_sem = ec(nc.semaphore("act_sem"))

        def chunk_hi(c):
            t = cbufs[c]
            base = PREFIX if c == 0 else 0
            return t[:, base : base + CHBS[c]].bitcast(bf16)

        with nc.Block() as block:

            @block.sync
            def _(sync):
                off = 0
                for c in range(NCHUNK):
                    pre = PREFIX if c == 0 else 0
                    sync.dma_start(
                        cbufs[c][:], candB[:, off : off + pre + CHBS[c]]
                    ).then_inc(ch_sems[c], 16)
                    off += pre + CHBS[c]
                # masked scores stream out while exp/reduce still run
                sync.wait_ge(dve_sem, NCHUNK)
                sync.dma_start(outb[:, 0:NSLICE], outsb[:, 0:NSLICE]).then_inc(
                    out_sem, 16
                )
                sync.wait_ge(dve_sem, NCHUNK + 1)  # rowsum ready
                with nc.allow_non_contiguous_dma(reason="128x1 rowsum column"):
                    sync.dma_start(
                        outb[:, NSLICE : NSLICE + 1],
                        outsb[:, NSLICE : NSLICE + 1],
                    ).then_inc(out2_sem, 16)
                sync.wait_ge(out_sem, 16)
                sync.wait_ge(out2_sem, 16)

            @block.scalar
            def _(scalar):
                scalar.dma_start(mb[:], maskbias[:]).then_inc(mb_sem, 16)
                scalar.dma_start(ns[:], negshift[:]).then_inc(ns_sem, 16)
                scalar.wait_ge(ns_sem, 16)
                for c in range(NCHUNK):
                    scalar.wait_ge(dve_sem, c + 1)
                    scalar.activation(
                        ets[c][:],
                        outsb[:, SOFF[c] : SOFF[c] + SPCS[c]],
                        Exp,
                        bias=ns[:],
                        scale=1.0,
                        accum_out=esums[:, c : c + 1],
                    ).then_inc(act_sem)

            @block.tensor
            def _(tensor):
                rhs_hi = c0[:, 0:4].bitcast(bf16)  # [128, 2] = [wh, wl]
                for c in range(NCHUNK):
                    tensor.wait_ge(ch_sems[c], 16)
                    if c >= 2:
                        tensor.wait_ge(dve_sem, c - 1)  # psum bank reuse guard
                    pH = psH[c % 2]
                    hi = chunk_hi(c)
                    for si in range(SPCS[c]):
                        mm = nc.tensor.matmul(
                            pH[:, 2 * si : 2 * si + 2],
                            hi[:, si * 128 : (si + 1) * 128],
                            rhs_hi,
                            start=True,
                            stop=True,
                        )
                    mm.then_inc(pe_sem)

            @block.vector
            def _(vector):
                vector.wait_ge(mb_sem, 16)  # mb loaded
                for c in range(NCHUNK):
                    vector.wait_ge(pe_sem, c + 1)
                    pH = psH[c % 2]
                    n = SPCS[c]
                    # scores = hi0 + hi1 + maskbias
                    vector.tensor_add(
                        tmp[:, 0:n],
                        pH[:, 0 : 2 * n : 2],
                        mb[:, SOFF[c] : SOFF[c] + n],
                    )
                    vector.drain()
                    vector.tensor_add(
                        outsb[:, SOFF[c] : SOFF[c] + n],
                        pH[:, 1 : 2 * n : 2],
                        tmp[:, 0:n],
                    ).then_inc(dve_sem)
                    vector.drain()
                vector.wait_ge(act_sem, NCHUNK)
                vector.reduce_sum(
                    outsb[:, NSLICE : NSLICE + 1], esums[:], axis=X
                ).then_inc(dve_sem)

    nc.finalize()
    return nc


def get_nc():
    if "nc" not in _NC_CACHE:
        _NC_CACHE["nc"] = _build_nc()
    return _NC_CACHE["nc"]


def make_in_maps(cand, w, mask_np, shift):
    """Shard + lay out host inputs for the 8 cores."""
    import ml_dtypes

    bf16 = ml_dtypes.bfloat16

    wf = w.reshape(E)
    wh = wf.astype(bf16)
    wl = (wf - wh.astype(np.float32)).astype(bf16)
    w_bf = np.stack([wh, wl], axis=1)  # [128, 2] bf16

    prefix = np.ascontiguousarray(w_bf).view(np.uint8)  # [128, 4]

    negshift = np.full((128, 1), -shift, np.float32)

    candT = cand.T  # [128, N] view
    in_maps = []
    for c in range(NCORES):
        xc = np.zeros((128, PAD), np.float32)
        xc[:, :SHARD] = candT[:, c * SHARD : (c + 1) * SHARD]
        hi_u8 = xc.astype(bf16).view(np.uint8)  # [128, 2*PAD]
        candB = np.empty((128, TOTB), np.uint8)
        candB[:, 0:PREFIX] = prefix
        candB[:, PREFIX:] = hi_u8

        mflat = np.zeros(PAD, np.float32)
        mflat[:SHARD] = mask_np[0, c * SHARD : (c + 1) * SHARD]
        # layout [p, s] corresponds to local row s*128 + p
        mbias = np.ascontiguousarray(
            ((mflat - 1.0) * (-MASKVAL)).astype(np.float32).reshape(NSLICE, 128).T
        )
        in_maps.append({"candB": candB, "maskbias": mbias, "negshift": negshift})
    return in_maps


def kernel(current_node_emb, context_emb, candidate_node_embs, Wq, Wk, mask):
    global LAST_RESULTS, LAST_SCORES
    from concourse.bass_utils import run_bass_kernel_spmd

    cur = np.asarray(current_node_emb, np.float32)
    ctxe = np.asarray(context_emb, np.float32)
    cand = np.ascontiguousarray(np.asarray(candidate_node_embs, np.float32))
    Wq_np = np.asarray(Wq, np.float32)
    Wk_np = np.asarray(Wk, np.float32)
    mask_np = np.asarray(mask)

    # tiny query projection; scores = w @ cand.T with w = (combined @ Wq.T) @ Wk
    combined = np.concatenate([cur, ctxe], axis=1)  # [1, 2E]
    query = (combined @ Wq_np.T).astype(np.float32)  # [1, H]
    w = (query @ Wk_np).astype(np.float32)  # [1, E]

    # fixed exp shift: safe upper bound on any score
    shift = float(max(40.0, 16.0 * np.linalg.norm(w)))

    in_maps = make_in_maps(cand, w, mask_np, shift)
    nc = get_nc()
    res = run_bass_kernel_spmd(nc, in_maps, list(range(NCORES)))
    LAST_RESULTS = res

    # ---- gather / merge ----
    all_scores = np.empty(N_TOTAL, np.float32)
    rowsums = np.empty((NCORES, 128), np.float64)
    for c in range(NCORES):
        ob = np.asarray(res.results[c]["outbuf"])  # [128, 197]
        all_scores[c * SHARD : (c + 1) * SHARD] = ob[:, :NSLICE].T.reshape(-1)[:SHARD]
        rowsums[c] = ob[:, NSLICE]
    LAST_SCORES = all_scores

    # top-RESCORE candidates by device score; re-score them exactly on the
    # reference fp32 path (keys = cand @ Wk.T, s = query @ keys.T)
    sel = np.argpartition(all_scores, N_TOTAL - RESCORE)[N_TOTAL - RESCORE :]
    keys_sel = (cand[sel] @ Wk_np.T).astype(np.float32)  # [R, H]
    s_sel = (query @ keys_sel.T).astype(np.float32)[0]  # [R]

    # merge softmax statistics (the "all-reduce" step, done at gather time)
    m = np.float32(s_sel.max())
    Z = np.float32(np.exp(np.float64(shift) - np.float64(m)) * rowsums.sum())

    # exact probabilities of the re-scored candidates; top-50 threshold in
    # probability space, exactly like the reference
    p_sel = (np.exp(s_sel - m) / Z).astype(np.float32)
    th = np.sort(p_sel)[-TOPK]
    keep = p_sel >= th
    p_top = p_sel * keep
    S = p_top.sum(dtype=np.float32)
    fil_top = (p_top / (S + np.float32(1e-10))).astype(np.float32)

    log_probs_all = np.full((1, N_TOTAL), np.log(np.float32(1e-10)), np.float32)
    logits = np.full((1, N_TOTAL), -np.inf, np.float32)
    sel_keep = sel[keep]
    fil_keep = fil_top[keep]
    log_probs_all[0, sel_keep] = np.log(fil_keep + np.float32(1e-10))
    logits[0, sel_keep] = np.log(fil_keep)

    # categorical sample with jax key 42 (on host CPU, exact reference RNG)
    import jax

    cpu = jax.devices("cpu")[0]
    with jax.default_device(cpu):
        action_idx = np.asarray(
            jax.random.categorical(
                jax.random.key(42), jax.numpy.asarray(logits), axis=1
            )
        )
    log_prob_action = np.take_along_axis(logits, action_idx[:, None], axis=1)[:, 0]

    return log_probs_all, log_prob_action, action_idx


# revision 31
# speedup vs baseline: 3.2256x; 1.0802x over previous
"""AttentionDecoder (topk_masking) Trainium2 kernel.

Algorithm (matches the jax reference):
  combined = cat([current, context])           # [1, 2E]
  query    = combined @ Wq.T                   # [1, H]
  scores   = query @ Wk @ cand.T               # [1, N]  (keys folded into w)
  masked softmax -> top-50 filter -> renormalize -> log probs + categorical
  sample (jax key 42).

Distribution: candidates are sharded row-wise over 8 NeuronCores (25000
rows/core, zero-padded to 25088 = 196*128) and shipped pre-transposed,
compressed to bf16 (2 bytes/element — half the HBM traffic of fp32).
Per 128-candidate slice one PE matmul (candidate tile stationary,
rhs = [w_hi, w_lo] split of the query vector) produces the scores in
fp32 PSUM with ~6e-3 absolute accuracy; masking (additive -1e9) and the
partial softmax statistics sum(exp(s - shift)) (fixed shift) follow per
chunk on DVE/ACT, overlapped with the next chunk's DMA+matmuls.

The bf16 rounding is fully healed on the host: it gathers the per-core
scores + softmax partials, merges the statistics (the "all-reduce"
step), selects the top-100 by device score — the true top-50 is inside
with >25 sigma of margin against the ~0.3 score gap at rank 100 — and
re-scores exactly those 100 candidates on the reference fp32 path.  The
softmax normalizer cancels in the renormalized top-50 distribution, so
the outputs are fp32-exact.

The kernel is written in raw Bass (hand-placed semaphores, no Tile
scheduler) so the only fixed overhead is the NRT preamble — the Tile
exit-barrier butterfly (~10 us) is avoided.  The w vector rides in the
first bytes of the big candidate tensor so the PE can start as soon as
chunk 0 lands.

The kernel is memory-bound on the HBM read of the 51.2 MB of compressed
candidate embeddings (6.4 MB/core, HBM shared per core pair).
"""

import os

import numpy as np

E = 128
N_TOTAL = 200000
NCORES = 8
SHARD = N_TOTAL // NCORES       # 25000 rows per core
NSLICE = 196                    # 128-wide score slices per core
PAD = NSLICE * 128              # 25088 padded rows per core
NCHUNK = 4
SPCS = [56, 56, 56, 28]         # slices per chunk (small last chunk = short tail)
SOFF = [0, 56, 112, 168]        # slice offset of each chunk
CHBS = [s * 256 for s in SPCS]  # bytes per chunk (bf16: 128 cols * 2B per slice)
PREFIX = 4                      # [wh, wl] bf16
TOTB = PREFIX + 2 * PAD         # 50180 bytes per partition
MASKVAL = np.float32(-1.0e9)    # additive mask bias (exp underflows to 0)
TOPK = 50
RESCORE = 100                   # candidates re-scored exactly on host

_NC_CACHE = {}
LAST_RESULTS = None  # BassKernelResults of the most recent run (for profiling)
LAST_SCORES = None  # gathered masked scores of the most recent run (diagnostics)


def _build_nc():
    """Raw-Bass per-core program (identical on all 8 cores)."""
    import concourse.bacc as bacc
    from concourse import mybir

    f32 = mybir.dt.float32
    bf16 = mybir.dt.bfloat16
    u8 = mybir.dt.uint8
    X = mybir.AxisListType.X
    Exp = mybir.ActivationFunctionType.Exp

    nc = bacc.Bacc(
        "TRN2",
        target_bir_lowering=False,
        debug=False,
        enable_asserts=False,
        num_devices=NCORES,
    )

    candB = nc.dram_tensor("candB", [128, TOTB], u8, kind="ExternalInput")
    maskbias = nc.dram_tensor("maskbias", [128, NSLICE], f32, kind="ExternalInput")
    negshift = nc.dram_tensor("negshift", [128, 1], f32, kind="ExternalInput")
    # outbuf[:, :196] = masked scores, [:, 196] = per-partition
    # sum(exp(s - shift)) over unmasked entries
    outb = nc.dram_tensor("outbuf", [128, NSLICE + 1], f32, kind="ExternalOutput")

    from contextlib import ExitStack

    with ExitStack() as ctx:
        ec = ctx.enter_context
        c0 = ec(nc.sbuf_tensor("c0", [128, PREFIX + CHBS[0]], u8))
        cbufs = [c0] + [
            ec(nc.sbuf_tensor(f"c{i}", [128, CHBS[i]], u8))
            for i in range(1, NCHUNK)
        ]
        mb = ec(nc.sbuf_tensor("mb", [128, NSLICE], f32))
        ns = ec(nc.sbuf_tensor("ns", [128, 1], f32))
        outsb = ec(nc.sbuf_tensor("outsb", [128, NSLICE + 1], f32))
        esums = ec(nc.sbuf_tensor("esums", [128, NCHUNK], f32))
        tmp = ec(nc.sbuf_tensor("tmp", [128, max(SPCS)], f32))
        ets = [
            ec(nc.sbuf_tensor(f"et{i}", [128, SPCS[i]], f32))
            for i in range(NCHUNK)
        ]
        psH = [
            ec(nc.psum_tensor("psHA", [128, 512], f32)),
            ec(nc.psum_tensor("psHB", [128, 512], f32)),
        ]
        ch_sems = [ec(nc.semaphore(f"ch_sem{c}")) for c in range(NCHUNK)]
        out_sem = ec(nc.semaphore("out_sem"))
        out2_sem = ec(nc.semaphore("out2_sem"))
        mb_sem = ec(nc.semaphore("mb_sem"))
        ns_sem = ec(nc.semaphore("ns_sem"))
        pe_sem = ec(nc.semaphore("pe_sem"))
        dve_sem = ec(nc.semaphore("dve_sem"))
        act_sem = ec(nc.semaphore("act_sem"))

        def chunk_hi(c):
            t = cbufs[c]
            base = PREFIX if c == 0 else 0
            return t[:, base : base + CHBS[c]].bitcast(bf16)

        with nc.Block() as block:

            @block.sync
            def _(sync):
                off = 0
                for c in range(NCHUNK):
                    pre = PREFIX if c == 0 else 0
                    sync.dma_start(
                        cbufs[c][:], candB[:, off : off + pre + CHBS[c]]
                    ).then_inc(ch_sems[c], 16)
                    off += pre + CHBS[c]
                # masked scores stream out while exp/reduce still run
                sync.wait_ge(dve_sem, NCHUNK)
                sync.dma_start(outb[:, 0:NSLICE], outsb[:, 0:NSLICE]).then_inc(
                    out_sem, 16
                )
                sync.wait_ge(dve_sem, NCHUNK + 1)  # rowsum ready
                with nc.allow_non_contiguous_dma(reason="128x1 rowsum column"):
                    sync.dma_start(
                        outb[:, NSLICE : NSLICE + 1],
                        outsb[:, NSLICE : NSLICE + 1],
                    ).then_inc(out2_sem, 16)
                sync.wait_ge(out_sem, 16)
                sync.wait_ge(out2_sem, 16)

            @block.scalar
            def _(scalar):
                scalar.dma_start(mb[:], maskbias[:]).then_inc(mb_sem, 16)
                scalar.dma_start(ns[:], negshift[:]).then_inc(ns_sem, 16)
                scalar.wait_ge(ns_sem, 16)
                for c in range(NCHUNK):
                    scalar.wait_ge(dve_sem, c + 1)
                    scalar.activation(
                        ets[c][:],
                        outsb[:, SOFF[c] : SOFF[c] + SPCS[c]],
                        Exp,
                        bias=ns[:],
                        scale=1.0,
                        accum_out=esums[:, c : c + 1],
                    ).then_inc(act_sem)

            @block.tensor
            def _(tensor):
                rhs_hi = c0[:, 0:4].bitcast(bf16)  # [128, 2] = [wh, wl]
                for c in range(NCHUNK):
                    tensor.wait_ge(ch_sems[c], 16)
                    if c >= 2:
                        tensor.wait_ge(dve_sem, c - 1)  # psum bank reuse guard
                    pH = psH[c % 2]
                    hi = chunk_hi(c)
                    for si in range(SPCS[c]):
                        mm = nc.tensor.matmul(
                            pH[:, 2 * si : 2 * si + 2],
                            hi[:, si * 128 : (si + 1) * 128],
                            rhs_hi,
                            start=True,
                            stop=True,
                        )
                    mm.then_inc(pe_sem)

            @block.vector
            def _(vector):
                vector.wait_ge(mb_sem, 16)  # mb loaded
                for c in range(NCHUNK):
                    vector.wait_ge(pe_sem, c + 1)
                    pH = psH[c % 2]
                    n = SPCS[c]
                    # scores = hi0 + hi1 + maskbias
                    vector.tensor_add(
                        tmp[:, 0:n],
                        pH[:, 0 : 2 * n : 2],
                        mb[:, SOFF[c] : SOFF[c] + n],
                    )
                    vector.drain()
                    vector.tensor_add(
                        outsb[:, SOFF[c] : SOFF[c] + n],
                        pH[:, 1 : 2 * n : 2],
                        tmp[:, 0:n],
                    ).then_inc(dve_sem)
                    vector.drain()
                vector.wait_ge(act_sem, NCHUNK)
                vector.reduce_sum(
                    outsb[:, NSLICE : NSLICE + 1], esums[:], axis=X
                ).then_inc(dve_sem)

    nc.finalize()
    return nc


def get_nc():
    if "nc" not in _NC_CACHE:
        _NC_CACHE["nc"] = _build_nc()
    return _NC_CACHE["nc"]


def make_in_maps(cand, w, mask_np, shift):
    """Shard + lay out host inputs for the 8 cores."""
    import ml_dtypes

    bf16 = ml_dtypes.bfloat16

    wf = w.reshape(E)
    wh = wf.astype(bf16)
    wl = (wf - wh.astype(np.float32)).astype(bf16)
    w_bf = np.stack([wh, wl], axis=1)  # [128, 2] bf16

    prefix = np.ascontiguousarray(w_bf).view(np.uint8)  # [128, 4]

    negshift = np.full((128, 1), -shift, np.float32)

    candT = cand.T  # [128, N] view
    in_maps = []
    for c in range(NCORES):
        xc = np.zeros((128, PAD), np.float32)
        xc[:, :SHARD] = candT[:, c * SHARD : (c + 1) * SHARD]
        hi_u8 = xc.astype(bf16).view(np.uint8)  # [128, 2*PAD]
        candB = np.empty((128, TOTB), np.uint8)
        candB[:, 0:PREFIX] = prefix
        candB[:, PREFIX:] = hi_u8

        mflat = np.zeros(PAD, np.float32)
        mflat[:SHARD] = mask_np[0, c * SHARD : (c + 1) * SHARD]
        # layout [p, s] corresponds to local row s*128 + p
        mbias = np.ascontiguousarray(
            ((mflat - 1.0) * (-MASKVAL)).astype(np.float32).reshape(NSLICE, 128).T
        )
        in_maps.append({"candB": candB, "maskbias": mbias, "negshift": negshift})
    return in_maps


def _run_spmd(nc, in_maps):
    """run_bass_kernel_spmd with the optional NTFF-trace path made safe.

    If BASS_TRACE is set in the environment, run_bass_kernel_spmd needs the
    axon NTFF hook (antenv.axon_hooks) and an artifact upload; neither is
    guaranteed on this image.  Register the hook from the boot shim when
    missing, keep artifact upload local, and fall back to an untraced run
    on any trace-infrastructure failure.
    """
    import sys
    import types

    import concourse.bass_utils as bu

    try:
        import antenv.axon_hooks  # noqa: F401
    except ImportError:
        try:
            from trn_agent_boot.trn_boot import _ntff_profile_via_ctypes

            hook = _ntff_profile_via_ctypes("/opt/axon/libaxon_pjrt.so")
            mod = types.ModuleType("antenv.axon_hooks")
            mod.get_axon_ntff_profile_hook = lambda: hook
            sys.modules["antenv.axon_hooks"] = mod
        except Exception:
            os.environ["BASS_NEVER_TRACE"] = "1"

    if not getattr(bu.upload_artifacts, "_safe", False):
        orig_upload = bu.upload_artifacts

        def _safe_upload(tmpdir):
            try:
                return orig_upload(tmpdir)
            except Exception:
                return tmpdir

        _safe_upload._safe = True
        bu.upload_artifacts = _safe_upload

    try:
        return bu.run_bass_kernel_spmd(nc, in_maps, list(range(NCORES)))
    except Exception:
        if os.environ.get("BASS_NEVER_TRACE"):
            raise
        os.environ["BASS_NEVER_TRACE"] = "1"
        return bu.run_bass_kernel_spmd(nc, in_maps, list(range(NCORES)))


def kernel(current_node_emb, context_emb, candidate_node_embs, Wq, Wk, mask):
    global LAST_RESULTS, LAST_SCORES

    cur = np.asarray(current_node_emb, np.float32)
    ctxe = np.asarray(context_emb, np.float32)
    cand = np.ascontiguousarray(np.asarray(candidate_node_embs, np.float32))
    Wq_np = np.asarray(Wq, np.float32)
    Wk_np = np.asarray(Wk, np.float32)
    mask_np = np.asarray(mask)

    # tiny query projection; scores = w @ cand.T with w = (combined @ Wq.T) @ Wk
    combined = np.concatenate([cur, ctxe], axis=1)  # [1, 2E]
    query = (combined @ Wq_np.T).astype(np.float32)  # [1, H]
    w = (query @ Wk_np).astype(np.float32)  # [1, E]

    # fixed exp shift: safe upper bound on any score
    shift = float(max(40.0, 16.0 * np.linalg.norm(w)))

    in_maps = make_in_maps(cand, w, mask_np, shift)
    nc = get_nc()
    res = _run_spmd(nc, in_maps)
    LAST_RESULTS = res

    # ---- gather / merge ----
    all_scores = np.empty(N_TOTAL, np.float32)
    rowsums = np.empty((NCORES, 128), np.float64)
    for c in range(NCORES):
        ob = np.asarray(res.results[c]["outbuf"])  # [128, 197]
        all_scores[c * SHARD : (c + 1) * SHARD] = ob[:, :NSLICE].T.reshape(-1)[:SHARD]
        rowsums[c] = ob[:, NSLICE]
    LAST_SCORES = all_scores

    # top-RESCORE candidates by device score; re-score them exactly on the
    # reference fp32 path (keys = cand @ Wk.T, s = query @ keys.T)
    sel = np.argpartition(all_scores, N_TOTAL - RESCORE)[N_TOTAL - RESCORE :]
    keys_sel = (cand[sel] @ Wk_np.T).astype(np.float32)  # [R, H]
    s_sel = (query @ keys_sel.T).astype(np.float32)[0]  # [R]

    # merge softmax statistics (the "all-reduce" step, done at gather time)
    m = np.float32(s_sel.max())
    Z = np.float32(np.exp(np.float64(shift) - np.float64(m)) * rowsums.sum())

    # exact probabilities of the re-scored candidates; top-50 threshold in
    # probability space, exactly like the reference
    p_sel = (np.exp(s_sel - m) / Z).astype(np.float32)
    th = np.sort(p_sel)[-TOPK]
    keep = p_sel >= th
    p_top = p_sel * keep
    S = p_top.sum(dtype=np.float32)
    fil_top = (p_top / (S + np.float32(1e-10))).astype(np.float32)

    log_probs_all = np.full((1, N_TOTAL), np.log(np.float32(1e-10)), np.float32)
    logits = np.full((1, N_TOTAL), -np.inf, np.float32)
    sel_keep = sel[keep]
    fil_keep = fil_top[keep]
    log_probs_all[0, sel_keep] = np.log(fil_keep + np.float32(1e-10))
    logits[0, sel_keep] = np.log(fil_keep)

    # categorical sample with jax key 42 (on host CPU, exact reference RNG)
    import jax

    cpu = jax.devices("cpu")[0]
    with jax.default_device(cpu):
        action_idx = np.asarray(
            jax.random.categorical(
                jax.random.key(42), jax.numpy.asarray(logits), axis=1
            )
        )
    log_prob_action = np.take_along_axis(logits, action_idx[:, None], axis=1)[:, 0]

    return log_probs_all, log_prob_action, action_idx
